# revision 1
# baseline (speedup 1.0000x reference)
"""Trainium2 Bass kernel for nn_AttentionModel (sparse_attention).

8-core distribution:
 - layer-1 convs: tensor-parallel over output channels (128/core), full x input.
   Outputs stay LOCAL (no gather).
 - layer-2 convs: each core computes PARTIAL sums for ALL output channels from
   its local 128-channel stage-1 slice; a ReduceScatter sums the partials and
   hands each core its output-channel shard (so q2 runs at M=128 instead of 32).
 - layer-3 convs: channel-sharded over an AllGather of the layer-2 shards.
 - attention tail: scores+softmax replicated; o and the 1x1 projection are
   POSITION-sharded (each core owns 256 of 2048 query positions), so no big
   gather of o is needed. The per-core beta column-slice is fetched with an
   indirect DMA driven by a per-core index input (keeps the program SPMD).

dtypes: convs/o/proj in float32r (1 cyc/row at N>=512), fp32 PSUM accumulation,
ReduceScatter in fp32; scores matmul + softmax in fp32.
"""
import os
import sys
import numpy as np

for _p in ('/opt/trn_rl_repo', '/root/problem/work'):
    if _p not in sys.path:
        sys.path.insert(0, _p)

import concourse.bass as bass
import concourse.bacc as bacc
import concourse.tile as tile
import concourse.mybir as mybir
from concourse import bass_utils
from concourse.bass_interp import get_hw_module

F32 = mybir.dt.float32
F32R = mybir.dt.float32r
I32 = mybir.dt.int32
AF = mybir.ActivationFunctionType
ALU = mybir.AluOpType
AX = mybir.AxisListType

NCORES = 8
NPOS = 2048
_CACHE = {}


def _lrelu(nc, sb, src_ap, bias_ap, bias3_ap, out_ap, name):
    """out = max(src + b, 0.3*src + 0.3b)  (LeakyReLU 0.3; HW Lrelu ignores alpha).
    Processed in <=1024-wide chunks of a flattened free dim to bound temp size."""
    P = src_ap.shape[0]
    free = int(np.prod(src_ap.shape[1:]))
    if len(src_ap.shape) == 2 and free > 1024:
        for lo in range(0, free, 1024):
            hi = min(lo + 1024, free)
            _lrelu(nc, sb, src_ap[:, lo:hi], bias_ap, bias3_ap, out_ap[:, lo:hi],
                   f"{name}_{lo}")
        return
    s = sb.tile([P, free], F32, name=f"{name}_s", tag="epi_s")
    t = sb.tile([P, free], F32, name=f"{name}_t", tag="epi_t")
    nc.scalar.activation(s[:], src_ap, AF.Identity, bias=bias_ap, scale=1.0)
    nc.scalar.activation(t[:], src_ap, AF.Identity, bias=bias3_ap, scale=0.3)
    nc.vector.tensor_tensor(out_ap, s[:], t[:], op=ALU.max)


def build_program():
    nc = bacc.Bacc("TRN2", target_bir_lowering=False, debug=False,
                   enable_asserts=True, num_devices=NCORES)

    xpad_d = nc.dram_tensor("xpad", [16, 128, 34 * 66], F32, kind="ExternalInput")
    xdec_d = nc.dram_tensor("xdec", [16, 128, 4 * 17 * 33], F32, kind="ExternalInput")
    w1q_d = nc.dram_tensor("w1q", [16, 128, 1152], F32, kind="ExternalInput")
    w1k_d = nc.dram_tensor("w1k", [16, 128, 1152], F32, kind="ExternalInput")
    w1v_d = nc.dram_tensor("w1v", [16, 128, 1152], F32, kind="ExternalInput")
    w2q_d = nc.dram_tensor("w2q", [128, 2304], F32, kind="ExternalInput")
    w2k_d = nc.dram_tensor("w2k", [128, 2304], F32, kind="ExternalInput")
    w2v_d = nc.dram_tensor("w2v", [2, 128, 4608], F32, kind="ExternalInput")
    w3q_d = nc.dram_tensor("w3q", [2, 128, 288], F32, kind="ExternalInput")
    w3k_d = nc.dram_tensor("w3k", [2, 128, 288], F32, kind="ExternalInput")
    w3v_d = nc.dram_tensor("w3v", [8, 128, 1152], F32, kind="ExternalInput")
    wp_d = nc.dram_tensor("wp", [8, 128, 1024], F32, kind="ExternalInput")
    bias_d = nc.dram_tensor("bias", [128, 28], F32, kind="ExternalInput")
    bidx_d = nc.dram_tensor("bidx", [65, 1], I32, kind="ExternalInput")
    out_d = nc.dram_tensor("out_shard", [1024, 256], F32, kind="ExternalOutput")
    ident_d = nc.inline_tensor(np.eye(128, dtype=np.float32), name="ident")

    RG = [list(range(NCORES))]

    with tile.TileContext(nc) as tc:
        with (
            tc.tile_pool(name="dram", bufs=1, space="DRAM") as dram,
            tc.tile_pool(name="wpool", bufs=2) as wpool,
            tc.tile_pool(name="xpool", bufs=2) as xpool,
            tc.tile_pool(name="opool", bufs=1) as opool,
            tc.tile_pool(name="ppool", bufs=1, space="PSUM") as ppool,
            tc.tile_pool(name="misc", bufs=1) as misc,
        ):
            # collective buffers
            rsa_in = dram.tile([8, 17920], F32)                  # k2/v2 partials
            rsa_out = dram.tile([17920], F32)
            rsb_in = dram.tile([8, 65536], F32)                  # q2 partials
            rsb_out = dram.tile([65536], F32)
            ag2a_in = dram.tile([17920], F32R)                   # k2/v2 shards
            ag2a_out = dram.tile([8, 17920], F32R, addr_space="Shared")
            ag2b_in = dram.tile([32, 2244], F32R)                # q2 shard (padded)
            ag2b_out = dram.tile([256, 2244], F32R, addr_space="Shared")
            ag3_in = dram.tile([32, 2373], F32)                  # q3 | k3 | v3
            ag3_out = dram.tile([256, 2373], F32, addr_space="Shared")
            beta_dram = dram.tile([65, 2048], F32)

            biases = misc.tile([128, 28], F32)
            nc.sync.dma_start(biases[:], bias_d.ap())
            bcol = lambda j: biases[:, j:j + 1]

            # tiny warmup collective: pays the first-collective setup cost
            # while stage 1 computes
            warm_in = dram.tile([128, 4], F32)
            warm_out = dram.tile([1024, 4], F32, addr_space="Shared")
            nc.sync.dma_start(warm_in[:], bias_d.ap()[:, 0:4])
            nc.gpsimd.collective_compute("AllGather", ALU.bypass, replica_groups=RG,
                                         ins=[warm_in.opt()], outs=[warm_out.opt()])

            # ============ STAGE 1: layer-1 convs (single pass, PE-bound) =====
            q1_ps = ppool.tile([128, 2048], F32, name="q1_ps", tag="pbig")
            k1_ps = ppool.tile([128, 512], F32, name="k1_ps", tag="pk")
            v1_ps = ppool.tile([128, 512], F32, name="v1_ps", tag="pv")
            for ic in range(16):
                xp = xpool.tile([128, 34 * 66], F32R, name="xp", tag="xbig")
                nc.gpsimd.dma_start(xp[:], xpad_d.ap()[ic])     # cast f32 -> f32r
                xd = xpool.tile([128, 4 * 17 * 33], F32R, name="xd", tag="xdec")
                nc.gpsimd.dma_start(xd[:], xdec_d.ap()[ic])
                wq = wpool.tile([128, 1152], F32R, name="wq", tag="wA")
                nc.gpsimd.dma_start(wq[:], w1q_d.ap()[ic])
                wk = wpool.tile([128, 1152], F32R, name="wk", tag="wB")
                nc.gpsimd.dma_start(wk[:], w1k_d.ap()[ic])
                wv = wpool.tile([128, 1152], F32R, name="wv", tag="wC")
                nc.gpsimd.dma_start(wv[:], w1v_d.ap()[ic])
                x3 = xp.rearrange("c (h w) -> c h w", h=34)
                xd4 = xd.rearrange("c (f h w) -> c f h w", f=4, h=17)
                first, last = (ic == 0), (ic == 15)
                for tap in range(9):
                    dy, dx = tap // 3, tap % 3
                    wq_t = wq[:, tap * 128:tap * 128 + 128]
                    for t in range(4):
                        win = x3[:, 8 * t + dy: 8 * t + dy + 8, dx: dx + 64]
                        nc.tensor.matmul(q1_ps[:, 512 * t: 512 * t + 512], wq_t, win,
                                         start=(first and tap == 0),
                                         stop=(last and tap == 8))
                for tap in range(9):
                    dy, dx = tap // 3, tap % 3
                    ph = 2 * (dy % 2) + (dx % 2)
                    win2 = xd4[:, ph:ph + 1, dy // 2: dy // 2 + 16, dx // 2: dx // 2 + 32]
                    nc.tensor.matmul(k1_ps[:], wk[:, tap * 128:tap * 128 + 128], win2,
                                     start=(first and tap == 0), stop=(last and tap == 8))
                    nc.tensor.matmul(v1_ps[:], wv[:, tap * 128:tap * 128 + 128], win2,
                                     start=(first and tap == 0), stop=(last and tap == 8))

            k1_sb = opool.tile([128, 15 * 33], F32R, name="k1_sb", tag="okv")
            k1o = k1_sb.rearrange("c (h w) -> c h w", h=15)
            k1g = k1_ps.rearrange("c (h w) -> c h w", h=16)
            _lrelu(nc, misc, k1g[:, 0:15, 0:31], bcol(1), bcol(11), k1o[:, :, 0:31], "k1e")
            nc.vector.tensor_copy(k1o[:, :, 31:33], k1o[:, :, 0:2])
            v1_sb = opool.tile([128, 15 * 33], F32R, name="v1_sb", tag="ovv")
            v1o = v1_sb.rearrange("c (h w) -> c h w", h=15)
            v1g = v1_ps.rearrange("c (h w) -> c h w", h=16)
            _lrelu(nc, misc, v1g[:, 0:15, 0:31], bcol(2), bcol(12), v1o[:, :, 0:31], "v1e")
            nc.vector.tensor_copy(v1o[:, :, 31:33], v1o[:, :, 0:2])

            q1_sb = opool.tile([128, 34 * 66], F32R, name="q1_sb", tag="obig")
            q1o = q1_sb.rearrange("c (h w) -> c h w", h=34)
            q1v = q1_ps.rearrange("c (h w) -> c h w", h=32)
            _lrelu(nc, misc, q1v[:, 0:16, :], bcol(0), bcol(10), q1o[:, 1:17, 1:65], "q1e0")
            _lrelu(nc, misc, q1v[:, 16:32, :], bcol(0), bcol(10), q1o[:, 17:33, 1:65], "q1e1")
            nc.vector.tensor_copy(q1o[:, 0:1, 1:65], q1o[:, 2:3, 1:65])
            nc.vector.tensor_copy(q1o[:, 33:34, 1:65], q1o[:, 31:32, 1:65])
            nc.vector.tensor_copy(q1o[:, :, 0:1], q1o[:, :, 64:65])
            nc.vector.tensor_copy(q1o[:, :, 65:66], q1o[:, :, 1:2])


            # k2/v2 partial convs + RSa, issued now so they overlap the q2 partials
            w2k = opool.tile([128, 2304], F32R, name="w2k", tag="wk2")
            nc.gpsimd.dma_start(w2k[:], w2k_d.ap())
            for cc in range(2):
                kp = ppool.tile([128, 112], F32, name="kp", tag="pk")
                for tap in range(9):
                    dy, dx = tap // 3, tap % 3
                    wink = k1o[:, dy: dy + 13: 2, dx: dx + 31: 2]
                    nc.tensor.matmul(kp[:], w2k[:, tap * 256 + 128 * cc: tap * 256 + 128 * cc + 128],
                                     wink, start=(tap == 0), stop=(tap == 8))
                kps = misc.tile([128, 112], F32, name="kps", tag="rss")
                nc.scalar.copy(kps[:], kp[:])
                dst = rsa_in[4 * cc:4 * cc + 4, 0:3584].rearrange("r (c p) -> r c p", c=32)
                nc.sync.dma_start(dst, kps[:])
            for vh in range(2):
                w2v = opool.tile([128, 4608], F32R, name="w2v", tag="wbig")
                nc.gpsimd.dma_start(w2v[:], w2v_d.ap()[vh])
                for cc4 in range(4):
                    cc = 4 * vh + cc4
                    vp = ppool.tile([128, 112], F32, name="vp", tag="pv")
                    for tap in range(9):
                        dy, dx = tap // 3, tap % 3
                        winv = v1o[:, dy: dy + 13: 2, dx: dx + 31: 2]
                        nc.tensor.matmul(vp[:], w2v[:, tap * 512 + 128 * cc4: tap * 512 + 128 * cc4 + 128],
                                         winv, start=(tap == 0), stop=(tap == 8))
                    vps = misc.tile([128, 112], F32, name="vps", tag="rss")
                    nc.scalar.copy(vps[:], vp[:])
                    nc.sync.dma_start(rsa_in[cc, 3584:17920].rearrange("(c p) -> c p", c=128), vps[:])
            nc.gpsimd.collective_compute("ReduceScatter", ALU.add, replica_groups=RG,
                                         ins=[rsa_in.opt()], outs=[rsa_out.opt()])

            # k2/v2 shard epilogues + AG2a (all overlap the q1 pass below)
            k2r = misc.tile([32, 112], F32, name="k2r", tag="rsl")
            nc.sync.dma_start(k2r[:], rsa_out[0:3584].rearrange("(c p) -> c p", c=32))
            v2r = misc.tile([128, 112], F32, name="v2r", tag="rsl2")
            nc.sync.dma_start(v2r[:], rsa_out[3584:17920].rearrange("(c p) -> c p", c=128))
            k2_sb = opool.tile([32, 112], F32R, name="k2_sb", tag="okv2")
            k2o = k2_sb.rearrange("c (h w) -> c h w", h=7)
            k2rg = k2r.rearrange("c (h w) -> c h w", h=7)
            _lrelu(nc, misc, k2rg[:, :, 0:15], bcol(4)[0:32], bcol(14)[0:32], k2o[:, :, 0:15], "k2e")
            nc.vector.tensor_copy(k2o[:, :, 15:16], k2o[:, :, 0:1])
            v2_sb = opool.tile([128, 112], F32R, name="v2_sb", tag="ovv2")
            v2o = v2_sb.rearrange("c (h w) -> c h w", h=7)
            v2rg = v2r.rearrange("c (h w) -> c h w", h=7)
            _lrelu(nc, misc, v2rg[:, :, 0:15], bcol(5), bcol(15), v2o[:, :, 0:15], "v2e")
            nc.vector.tensor_copy(v2o[:, :, 15:16], v2o[:, :, 0:1])
            nc.sync.dma_start(ag2a_in[0:3584].rearrange("(c p) -> c p", c=32), k2_sb[:])
            nc.sync.dma_start(ag2a_in[3584:17920].rearrange("(c p) -> c p", c=128), v2_sb[:])
            nc.gpsimd.collective_compute("AllGather", ALU.bypass, replica_groups=RG,
                                         ins=[ag2a_in.opt()], outs=[ag2a_out.opt()])

            # q2 partials (M=128!) from local q1
            w2q = opool.tile([128, 2304], F32R, name="w2q", tag="wq2")
            nc.gpsimd.dma_start(w2q[:], w2q_d.ap())
            for cc in range(2):
                qp = ppool.tile([128, 2048], F32, name="qp", tag="pbig")
                for tap in range(9):
                    dy, dx = tap // 3, tap % 3
                    wslc = w2q[:, tap * 256 + 128 * cc: tap * 256 + 128 * cc + 128]
                    for t in range(4):
                        win = q1o[:, 8 * t + dy: 8 * t + dy + 8, dx: dx + 64]
                        nc.tensor.matmul(qp[:, 512 * t:512 * t + 512], wslc, win,
                                         start=(tap == 0), stop=(tap == 8))
                qps = misc.tile([128, 2048], F32, name="qps", tag="rssb")
                nc.scalar.copy(qps[:], qp[:])
                dst = rsb_in[4 * cc:4 * cc + 4, :].rearrange("r (c p) -> r c p", c=32)
                nc.sync.dma_start(dst, qps[:])
            nc.gpsimd.collective_compute("ReduceScatter", ALU.add, replica_groups=RG,
                                         ins=[rsb_in.opt()], outs=[rsb_out.opt()])

            q2r = misc.tile([32, 2048], F32, name="q2r", tag="rssb")
            nc.sync.dma_start(q2r[:], rsb_out[:].rearrange("(c p) -> c p", c=32))
            q2_sb = opool.tile([32, 34 * 66], F32R, name="q2_sb", tag="obig")
            q2o = q2_sb.rearrange("c (h w) -> c h w", h=34)
            q2rv = q2r.rearrange("c (h w) -> c h w", h=32)
            _lrelu(nc, misc, q2rv[:, 0:16, :], bcol(3)[0:32], bcol(13)[0:32], q2o[:, 1:17, 1:65], "q2e0")
            _lrelu(nc, misc, q2rv[:, 16:32, :], bcol(3)[0:32], bcol(13)[0:32], q2o[:, 17:33, 1:65], "q2e1")
            nc.vector.tensor_copy(q2o[:, 0:1, 1:65], q2o[:, 2:3, 1:65])
            nc.vector.tensor_copy(q2o[:, 33:34, 1:65], q2o[:, 31:32, 1:65])
            nc.vector.tensor_copy(q2o[:, :, 0:1], q2o[:, :, 64:65])
            nc.vector.tensor_copy(q2o[:, :, 65:66], q2o[:, :, 1:2])
            nc.sync.dma_start(ag2b_in[:], q2_sb[:])
            nc.gpsimd.collective_compute("AllGather", ALU.bypass, replica_groups=RG,
                                         ins=[ag2b_in.opt()], outs=[ag2b_out.opt()])

            # projection weights: load+convert early so they overlap stage 3
            wpts = []
            for half in range(2):
                wpt = opool.tile([128, 4096], F32R, name="wpt", tag=f"wpt{half}")
                for qtr in range(2):
                    wstg = misc.tile([128, 2048], F32, name="wstg", tag="wstg")
                    nc.sync.dma_start(wstg.rearrange("b (a c) -> b a c", a=2),
                                      wp_d.ap()[4 * half + 2 * qtr: 4 * half + 2 * qtr + 2].rearrange("a b c -> b a c"))
                    nc.vector.tensor_copy(wpt[:, 2048 * qtr:2048 * qtr + 2048], wstg[:])
                wpts.append(wpt)

            # ============ STAGE 3: k3/v3 first (need ag2a), then q3 ==========
            k3_ps = ppool.tile([32, 70], F32, name="k3_ps", tag="pk")
            v3_ps = ppool.tile([128, 70], F32, name="v3_ps", tag="pv")
            k2rr = ag2a_out[:, 0:3584].rearrange("r (c p) -> r c p", c=32)
            v2rr = ag2a_out[:, 3584:17920].rearrange("r (c p) -> r c p", c=128)
            for jc in range(2):
                k2c = xpool.tile([128, 112], F32R, name="k2c", tag="k2c")
                nc.sync.dma_start(k2c[:], k2rr[4 * jc:4 * jc + 4])
                k2c3 = k2c.rearrange("c (h w) -> c h w", h=7)
                w3k = wpool.tile([128, 288], F32R, name="w3k", tag="wB")
                nc.gpsimd.dma_start(w3k[:], w3k_d.ap()[jc])
                for tap in range(9):
                    dy, dx = tap // 3, tap % 3
                    # k3: out grid (5, 14), cols 0..12 valid; N=70
                    wink = k2c3[:, dy: dy + 5, dx: dx + 14]
                    nc.tensor.matmul(k3_ps[:], w3k[:, tap * 32:tap * 32 + 32], wink,
                                     start=(jc == 0 and tap == 0), stop=(jc == 1 and tap == 8))
            for ic in range(8):
                v2c = xpool.tile([128, 112], F32R, name="v2c", tag="v2c")
                nc.sync.dma_start(v2c[:], v2rr[ic])
                v2c3 = v2c.rearrange("c (h w) -> c h w", h=7)
                w3v = wpool.tile([128, 1152], F32R, name="w3v", tag="wC")
                nc.gpsimd.dma_start(w3v[:], w3v_d.ap()[ic])
                for tap in range(9):
                    dy, dx = tap // 3, tap % 3
                    winv = v2c3[:, dy: dy + 5, dx: dx + 14]
                    nc.tensor.matmul(v3_ps[:], w3v[:, tap * 128:tap * 128 + 128], winv,
                                     start=(ic == 0 and tap == 0), stop=(ic == 7 and tap == 8))

            k3g = k3_ps.rearrange("c (h w) -> c h w", h=5)
            k3_sb = opool.tile([32, 65], F32, name="k3_sb", tag="okv")
            _lrelu(nc, misc, k3g[:, :, 0:13], bcol(7)[0:32], bcol(17)[0:32], k3_sb[:], "k3e")
            v3g = v3_ps.rearrange("c (h w) -> c h w", h=5)
            v3_sb = opool.tile([128, 65], F32, name="v3_sb", tag="v3sb")
            _lrelu(nc, misc, v3g[:, :, 0:13], bcol(8), bcol(18), v3_sb[:], "v3e")

            q3_ps = ppool.tile([32, 2048], F32, name="q3_ps", tag="pbig")
            for jc in range(2):
                q2p_t = xpool.tile([128, 34 * 66], F32R, name="q2p", tag="xbig")
                nc.sync.dma_start(q2p_t[:], ag2b_out[128 * jc:128 * jc + 128])
                q2p = q2p_t.rearrange("c (h w) -> c h w", h=34)
                w3q = wpool.tile([128, 288], F32R, name="w3q", tag="wA")
                nc.gpsimd.dma_start(w3q[:], w3q_d.ap()[jc])
                first, last = (jc == 0), (jc == 1)
                for tap in range(9):
                    dy, dx = tap // 3, tap % 3
                    for t in range(4):
                        win = q2p[:, 8 * t + dy: 8 * t + dy + 8, dx: dx + 64]
                        nc.tensor.matmul(q3_ps[:, 512 * t:512 * t + 512],
                                         w3q[:, tap * 32:tap * 32 + 32], win,
                                         start=(first and tap == 0), stop=(last and tap == 8))

            q3_sb = opool.tile([32, 2048], F32, name="q3_sb", tag="obig")
            _lrelu(nc, misc, q3_ps[:], bcol(6)[0:32], bcol(16)[0:32], q3_sb[:], "q3e")
            nc.sync.dma_start(ag3_in[:, 0:2048], q3_sb[:])
            nc.sync.dma_start(ag3_in[:, 2048:2113], k3_sb[:])
            # v3 packed as channel p -> (row p//4, col-block p%4)
            nc.sync.dma_start(ag3_in[:, 2113:2373].rearrange("c (a p) -> c a p", a=4), v3_sb[:])
            nc.gpsimd.collective_compute("AllGather", ALU.bypass, replica_groups=RG,
                                         ins=[ag3_in.opt()], outs=[ag3_out.opt()])

            # ============ STAGE 4: attention + position-sharded projection ===
            sc_ps = ppool.tile([65, 2048], F32, name="sc_ps", tag="pbig")
            for jc in range(2):
                q3f = opool.tile([128, 2048], F32, name="q3f", tag="wq2")
                nc.sync.dma_start(q3f[:], ag3_out[128 * jc:128 * jc + 128, 0:2048])
                k3f = misc.tile([128, 65], F32, name="k3f", tag="k3f", bufs=2)
                nc.sync.dma_start(k3f[:], ag3_out[128 * jc:128 * jc + 128, 2048:2113])
                for t in range(4):
                    nc.tensor.matmul(sc_ps[:, 512 * t:512 * t + 512], k3f[:],
                                     q3f[:, 512 * t:512 * t + 512],
                                     start=(jc == 0), stop=(jc == 1))

            negmax = misc.tile([65, 1], F32)
            nc.vector.reduce_max(negmax[:], sc_ps[:], axis=AX.X, negate=True)
            esum = misc.tile([65, 1], F32)
            bexp = misc.tile([65, 2048], F32)
            nc.scalar.activation(bexp[:], sc_ps[:], AF.Exp, bias=negmax[:, 0:1],
                                 accum_out=esum[:, 0:1])
            rsum = misc.tile([65, 1], F32)
            nc.vector.reciprocal(rsum[:], esum[:])
            # ship UNnormalized exp; the 1/sum(m) factor is folded into v3^T
            # below (commutes through the m-contraction of the o matmul)
            nc.sync.dma_start(beta_dram[:], bexp[:])


            # indirect gather of MY 256 beta columns: row (m, blk) of (520, 256)
            bidx = misc.tile([65, 1], I32)
            nc.sync.dma_start(bidx[:], bidx_d.ap())
            betaB = misc.tile([65, 256], F32)
            nc.gpsimd.indirect_dma_start(
                out=betaB[:], out_offset=None,
                in_=beta_dram.rearrange("m (b p) -> (m b) p", b=8),
                in_offset=bass.IndirectOffsetOnAxis(ap=bidx[:, 0:1], axis=0))

            # v3^T chunks (65, 128) for all 1024 v-channels
            ident = misc.tile([128, 128], F32)
            nc.sync.dma_start(ident[:], ident_d.ap())
            v3r = ag3_out[:, 2113:2373].rearrange("r (a p) -> r a p", a=4)
            v3ta = misc.tile([65, 1024], F32, name="v3ta")
            for i in range(8):
                v3f = misc.tile([128, 65], F32, name="v3f", tag="v3f", bufs=2)
                nc.sync.dma_start(v3f[:], v3r[32 * i:32 * i + 32])
                tps = ppool.tile([65, 128], F32, name="tps", tag="pk")
                nc.tensor.transpose(tps[:], v3f[:, 0:65], ident[:])
                nc.scalar.copy(v3ta[:, 128 * i:128 * i + 128], tps[:])
            nc.vector.tensor_scalar_mul(v3ta[:], v3ta[:], rsum[:, 0:1])

            # o chunks (128 v-ch, 256 pos) then projection (all 1024 out-ch)
            oia = misc.tile([128, 2048], F32R, name="oia", tag="rssb")
            for i in range(8):
                ops = ppool.tile([128, 256], F32, name="ops", tag="pv")
                nc.tensor.matmul(ops[:], v3ta[:, 128 * i:128 * i + 128], betaB[:],
                                 start=True, stop=True)
                nc.scalar.copy(oia[:, 256 * i:256 * i + 256], ops[:])
            acca = misc.tile([128, 2048], F32, name="acca", tag="bexp")
            for half in range(2):
                wpt = wpts[half]
                for cc in range(8):
                    out_ps = ppool.tile([128, 256], F32, name="out_ps", tag="pk")
                    for c4 in range(4):
                        cik = 4 * half + c4
                        nc.tensor.matmul(out_ps[:],
                                         wpt[:, 1024 * c4 + 128 * cc: 1024 * c4 + 128 * cc + 128],
                                         oia[:, 256 * cik:256 * cik + 256],
                                         start=(c4 == 0), stop=(c4 == 3))
                    if half == 0:
                        nc.vector.tensor_scalar_add(acca[:, 256 * cc:256 * cc + 256],
                                                    out_ps[:], bcol(20 + cc))
                    else:
                        out_sb = misc.tile([128, 256], F32, name="out_sb", tag="osb", bufs=2)
                        nc.vector.tensor_tensor(out_sb[:], acca[:, 256 * cc:256 * cc + 256],
                                                out_ps[:], op=ALU.add)
                        nc.sync.dma_start(out_d.ap()[128 * cc:128 * cc + 128], out_sb[:])

    nc.compile()
    nc.m = get_hw_module(nc.m)
    return nc


def _prep_inputs(x, qw1, qb1, qw2, qb2, qw3, qb3, kw1, kb1, kw2, kb2, kw3, kb3,
                 vw1, vb1, vw2, vb2, vw3, vb3, pw, pb):
    f = np.float32
    x = np.ascontiguousarray(np.asarray(x).reshape(2048, 32, 64), dtype=f)
    xp = np.concatenate([x[:, 1:2], x, x[:, 30:31]], axis=1)
    xp = np.concatenate([xp[:, :, -1:], xp, xp[:, :, :1]], axis=2)
    xpad = np.ascontiguousarray(xp.reshape(16, 128, 34 * 66))
    xdec = np.zeros((16, 128, 4, 17, 33), f)
    xr = x.reshape(16, 128, 32, 64)
    for py in range(2):
        for px in range(2):
            xdec[:, :, 2 * py + px, 0:16, 0:32] = xr[:, :, py::2, px::2]
    xdec = np.ascontiguousarray(xdec.reshape(16, 128, 4 * 17 * 33))

    def conv_w(wt, co_lo, co_n, nchunk):
        ws = np.asarray(wt)[co_lo:co_lo + co_n]           # (co_n, Ci, 3, 3)
        ci = ws.shape[1]
        a = ws.reshape(co_n, nchunk, ci // nchunk, 9)     # (co, ck, ci, tap)
        a = a.transpose(1, 2, 3, 0)                       # (ck, ci, tap, co)
        return np.ascontiguousarray(a.reshape(nchunk, ci // nchunk, 9 * co_n), dtype=f)

    def conv_w_ci(wt, ci_lo):
        # full out-channels, my 128 input channels -> (128ci, 9*co)
        ws = np.asarray(wt)[:, ci_lo:ci_lo + 128]         # (co, 128, 3, 3)
        co = ws.shape[0]
        a = ws.reshape(co, 128, 9).transpose(1, 2, 0)     # (ci, tap, co)
        return np.ascontiguousarray(a.reshape(128, 9 * co), dtype=f)

    in_maps = []
    for c in range(NCORES):
        m = {"xpad": xpad, "xdec": xdec}
        m["w1q"] = conv_w(qw1, 128 * c, 128, 16)
        m["w1k"] = conv_w(kw1, 128 * c, 128, 16)
        m["w1v"] = conv_w(vw1, 128 * c, 128, 16)
        m["w2q"] = conv_w_ci(qw2, 128 * c)
        m["w2k"] = conv_w_ci(kw2, 128 * c)
        wv2 = np.asarray(vw2)[:, 128 * c:128 * c + 128]        # (1024co, 128ci, 3, 3)
        wv2 = wv2.reshape(2, 512, 128, 9).transpose(0, 2, 3, 1)  # (half, ci, tap, co512)
        m["w2v"] = np.ascontiguousarray(wv2.reshape(2, 128, 4608), dtype=f)
        m["w3q"] = conv_w(qw3, 32 * c, 32, 2)
        m["w3k"] = conv_w(kw3, 32 * c, 32, 2)
        m["w3v"] = conv_w(vw3, 128 * c, 128, 8)
        m["wp"] = np.ascontiguousarray(
            np.asarray(pw)[:, :, 0, 0].T.reshape(8, 128, 1024), dtype=f)
        bias = np.zeros((128, 28), f)
        bias[:, 0] = qb1[128 * c:128 * c + 128]
        bias[:, 1] = kb1[128 * c:128 * c + 128]
        bias[:, 2] = vb1[128 * c:128 * c + 128]
        bias[0:32, 3] = qb2[32 * c:32 * c + 32]
        bias[0:32, 4] = kb2[32 * c:32 * c + 32]
        bias[:, 5] = vb2[128 * c:128 * c + 128]
        bias[0:32, 6] = qb3[32 * c:32 * c + 32]
        bias[0:32, 7] = kb3[32 * c:32 * c + 32]
        bias[:, 8] = vb3[128 * c:128 * c + 128]
        bias[:, 10:19] = 0.3 * bias[:, 0:9]
        for j in range(8):
            bias[:, 20 + j] = pb[128 * j:128 * j + 128]
        m["bias"] = bias
        m["bidx"] = np.arange(65, dtype=np.int32).reshape(65, 1) * 8 + c
        in_maps.append(m)
    return in_maps


LAST_RESULT = None


def kernel(**inputs):
    global LAST_RESULT
    if "nc" not in _CACHE:
        _CACHE["nc"] = build_program()
    nc = _CACHE["nc"]
    in_maps = _prep_inputs(**{k: np.asarray(v) for k, v in inputs.items()})
    res = bass_utils.run_bass_kernel_spmd(nc, in_maps, core_ids=list(range(NCORES)))
    LAST_RESULT = res
    out = np.empty((1024, 32, 64), np.float32)
    for c in range(NCORES):
        out[:, 4 * c:4 * c + 4, :] = res.results[c]["out_shard"].reshape(1024, 4, 64)
    return np.ascontiguousarray(out.reshape(1, 1024, 32, 64))



# revision 21
# speedup vs baseline: 1.3335x; 1.3335x over previous
"""Trainium2 Bass kernel for nn_AttentionModel (sparse_attention), v2.

8-core distribution (fp16 matmul inputs, fp32 PSUM accumulation):
 - layer-1 convs channel-TP (128 out-ch/core); the k/v branch runs FIRST so
   its whole collective chain hides under the big q1 conv.
 - k1/v1 read stride-2 windows straight from the padded x (no separate
   decimated copy of x).
 - k2/v2: local partials over the core's 128 k1/v1 channels -> one fp16
   AllReduce (Shared out) -> replicated epilogues -> channel-sharded k3/v3
   convs -> tiny k3|v3 AllGather.
 - q2: local partials for ALL 256 channels -> fp16 AllToAll whose slices are
   position blocks (4 H-rows/core + reflected halo rows baked in at send);
   the receiver does a local 8-way add. Everything after (q3, scores,
   softmax, o, 1x1 proj) is local to the core's 256 positions.
 - softmax over the query axis is globalized with a tiny (max, expsum)
   stats AllGather.
"""
import sys
import numpy as np

for _p in ('/opt/trn_rl_repo',):
    if _p not in sys.path:
        sys.path.insert(0, _p)

import concourse.bass as bass
import concourse.bacc as bacc
import concourse.tile as tile
import concourse.mybir as mybir
from concourse import bass_utils
from concourse.bass_interp import get_hw_module

F32 = mybir.dt.float32
F16 = mybir.dt.float16
AF = mybir.ActivationFunctionType
ALU = mybir.AluOpType
AX = mybir.AxisListType

NCORES = 8
_CACHE = {}


def _lrelu(nc, sb, src_ap, bias_ap, bias3_ap, out_ap, name):
    """out = max(src + b, 0.3*src + 0.3b)  (LeakyReLU 0.3)."""
    P = src_ap.shape[0]
    free = int(np.prod(src_ap.shape[1:]))
    s = sb.tile([P, free], F32, name=f"{name}_s", tag="epi_s")
    t = sb.tile([P, free], F32, name=f"{name}_t", tag="epi_t")
    nc.scalar.activation(s[:], src_ap, AF.Identity, bias=bias_ap, scale=1.0)
    nc.scalar.activation(t[:], src_ap, AF.Identity, bias=bias3_ap, scale=0.3)
    nc.vector.tensor_tensor(out_ap, s[:], t[:], op=ALU.max)


def build_program():
    nc = bacc.Bacc("TRN2", target_bir_lowering=False, debug=False,
                   enable_asserts=True, num_devices=NCORES)

    xpad_d = nc.dram_tensor("xpad", [16, 128, 34 * 66], F16, kind="ExternalInput")
    w1q_d = nc.dram_tensor("w1q", [16, 128, 1152], F16, kind="ExternalInput")
    w1k_d = nc.dram_tensor("w1k", [16, 128, 1152], F16, kind="ExternalInput")
    w1v_d = nc.dram_tensor("w1v", [16, 128, 1152], F16, kind="ExternalInput")
    w2q_d = nc.dram_tensor("w2q", [128, 2304], F16, kind="ExternalInput")
    w2k_d = nc.dram_tensor("w2k", [128, 2304], F16, kind="ExternalInput")
    w2v_d = nc.dram_tensor("w2v", [2, 128, 4608], F16, kind="ExternalInput")
    w3q_d = nc.dram_tensor("w3q", [2, 128, 2304], F16, kind="ExternalInput")
    w3k_d = nc.dram_tensor("w3k", [2, 128, 288], F16, kind="ExternalInput")
    w3v_d = nc.dram_tensor("w3v", [8, 128, 1152], F16, kind="ExternalInput")
    wp_d = nc.dram_tensor("wp", [8, 128, 1024], F16, kind="ExternalInput")
    bias_d = nc.dram_tensor("bias", [128, 46], F32, kind="ExternalInput")
    out_d = nc.dram_tensor("out_shard", [1024, 256], F32, kind="ExternalOutput")
    ident_d = nc.inline_tensor(np.eye(128, dtype=np.float16), name="ident")

    RG = [list(range(NCORES))]

    with tile.TileContext(nc) as tc:
        with (
            tc.tile_pool(name="dram", bufs=1, space="DRAM") as dram,
            tc.tile_pool(name="xres", bufs=1) as xres,
            tc.tile_pool(name="wpool", bufs=2) as wpool,
            tc.tile_pool(name="opool", bufs=1) as opool,
            tc.tile_pool(name="ppool", bufs=1, space="PSUM") as ppool,
            tc.tile_pool(name="misc", bufs=1) as misc,
        ):
            # collective buffers
            ar_in = dram.tile([134400], F16)                 # k2|v2 partials
            ar_out = dram.tile([134400], F16, addr_space="Shared")
            agkv_in = dram.tile([10400], F16)                # k3 | v3 shards
            agkv_out = dram.tile([8, 10400], F16, addr_space="Shared")
            a2a_in = dram.tile([8, 98304], F16)              # q2 partial slices
            a2a_out = dram.tile([8, 98304], F16)
            st_in = dram.tile([130], F32)                    # softmax stats
            st_out = dram.tile([8, 130], F32, addr_space="Shared")

            biases = misc.tile([128, 46], F32)
            nc.sync.dma_start(biases[:], bias_d.ap())
            bcol = lambda j: biases[:, j:j + 1]

            # tiny warmup collective: pays the first-collective setup cost
            warm_in = dram.tile([128, 4], F32)
            warm_out = dram.tile([1024, 4], F32, addr_space="Shared")
            nc.sync.dma_start(warm_in[:], bias_d.ap()[:, 0:4])
            nc.gpsimd.collective_compute("AllGather", ALU.bypass, replica_groups=RG,
                                         ins=[warm_in.opt()], outs=[warm_out.opt()])

            # x resident in SBUF: 16 chunks of [128, 34*66] fp16
            xall = xres.tile([128, 16 * 2244], F16, name="xall")
            for ic in range(16):
                nc.gpsimd.dma_start(xall[:, 2244 * ic:2244 * (ic + 1)],
                                    xpad_d.ap()[ic])

            # ============ k1/v1 convs (stride-2 windows from xpad) ===========
            k1_ps = ppool.tile([128, 465], F32, name="k1_ps", tag="pk")
            v1_ps = ppool.tile([128, 465], F32, name="v1_ps", tag="pv")
            for ic in range(16):
                wk = wpool.tile([128, 1152], F16, name="wk", tag="wB")
                nc.gpsimd.dma_start(wk[:], w1k_d.ap()[ic])
                wv = wpool.tile([128, 1152], F16, name="wv", tag="wC")
                nc.gpsimd.dma_start(wv[:], w1v_d.ap()[ic])
                x3 = xall[:, 2244 * ic:2244 * (ic + 1)].rearrange(
                    "c (h w) -> c h w", h=34)
                first, last = (ic == 0), (ic == 15)
                for tap in range(9):
                    dy, dx = tap // 3, tap % 3
                    win = x3[:, 1 + dy: 1 + dy + 29: 2, 1 + dx: 1 + dx + 61: 2]
                    nc.tensor.matmul(k1_ps[:], wk[:, tap * 128:tap * 128 + 128], win,
                                     start=(first and tap == 0), stop=(last and tap == 8))
                    nc.tensor.matmul(v1_ps[:], wv[:, tap * 128:tap * 128 + 128], win,
                                     start=(first and tap == 0), stop=(last and tap == 8))

            k1_sb = opool.tile([128, 465], F16, name="k1_sb", tag="okv")
            _lrelu(nc, misc, k1_ps[:], bcol(1), bcol(20), k1_sb[:], "k1e")
            v1_sb = opool.tile([128, 465], F16, name="v1_sb", tag="ovv")
            _lrelu(nc, misc, v1_ps[:], bcol(2), bcol(21), v1_sb[:], "v1e")
            k1o = k1_sb.rearrange("c (h w) -> c h w", h=15)
            v1o = v1_sb.rearrange("c (h w) -> c h w", h=15)

            # ============ k2/v2 partials + fp16 AllReduce ====================
            w2k = opool.tile([128, 2304], F16, name="w2k", tag="wk2")
            nc.gpsimd.dma_start(w2k[:], w2k_d.ap())
            kp = ppool.tile([128, 210], F32, name="kp", tag="pk")
            for cc in range(2):
                for tap in range(9):
                    dy, dx = tap // 3, tap % 3
                    wink = k1o[:, dy: dy + 13: 2, dx: dx + 29: 2]
                    nc.tensor.matmul(kp[:, 105 * cc:105 * cc + 105],
                                     w2k[:, tap * 256 + 128 * cc: tap * 256 + 128 * cc + 128],
                                     wink, start=(tap == 0), stop=(tap == 8))
            vp = ppool.tile([128, 1024], F32, name="vp", tag="pbig")
            for vh in range(2):
                w2v = wpool.tile([128, 4608], F16, name="w2v", tag="wv2")
                nc.gpsimd.dma_start(w2v[:], w2v_d.ap()[vh])
                for c4 in range(4):
                    cc = 4 * vh + c4
                    for tap in range(9):
                        dy, dx = tap // 3, tap % 3
                        winv = v1o[:, dy: dy + 13: 2, dx: dx + 29: 2]
                        nc.tensor.matmul(vp[:, 128 * cc:128 * cc + 105],
                                         w2v[:, tap * 512 + 128 * c4: tap * 512 + 128 * c4 + 128],
                                         winv, start=(tap == 0), stop=(tap == 8))
            kps = misc.tile([128, 210], F16, name="kps", tag="rsk")
            nc.scalar.copy(kps[:], kp[:])
            vps = misc.tile([128, 840], F16, name="vps", tag="rsv")
            nc.vector.tensor_copy(vps.rearrange("p (c f) -> p c f", c=8),
                                  vp.rearrange("p (c f) -> p c f", c=8)[:, :, 0:105])
            nc.sync.dma_start(ar_in[0:26880].rearrange("(c p f) -> p c f", c=2, p=128),
                              kps.rearrange("p (c f) -> p c f", c=2))
            nc.sync.dma_start(ar_in[26880:134400].rearrange("(c p f) -> p c f", c=8, p=128),
                              vps.rearrange("p (c f) -> p c f", c=8))
            nc.gpsimd.collective_compute("AllReduce", ALU.add, replica_groups=RG,
                                         ins=[ar_in.opt()], outs=[ar_out.opt()])

            # k2/v2 epilogues on the reduced FULL tensors (overlap q1)
            k2r = misc.tile([128, 210], F16, name="k2r", tag="rsk2")
            nc.sync.dma_start(k2r.rearrange("p (c f) -> p c f", c=2),
                              ar_out[0:26880].rearrange("(c p f) -> p c f", c=2, p=128))
            v2r = misc.tile([128, 840], F16, name="v2r", tag="rsv2")
            nc.sync.dma_start(v2r.rearrange("p (c f) -> p c f", c=8),
                              ar_out[26880:134400].rearrange("(c p f) -> p c f", c=8, p=128))
            k2_sb = opool.tile([128, 210], F16, name="k2_sb", tag="okv2")
            for cc in range(2):
                _lrelu(nc, misc, k2r[:, 105 * cc:105 * cc + 105], bcol(5 + cc),
                       bcol(24 + cc), k2_sb[:, 105 * cc:105 * cc + 105], f"k2e{cc}")
            v2_sb = opool.tile([128, 840], F16, name="v2_sb", tag="ovv2")
            for cc in range(8):
                _lrelu(nc, misc, v2r[:, 105 * cc:105 * cc + 105], bcol(7 + cc),
                       bcol(26 + cc), v2_sb[:, 105 * cc:105 * cc + 105], f"v2e{cc}")

            # ============ q1 conv ============================================
            q1_ps = ppool.tile([128, 2048], F32, name="q1_ps", tag="pbig")
            for ic in range(16):
                wq = wpool.tile([128, 1152], F16, name="wq", tag="wA")
                nc.gpsimd.dma_start(wq[:], w1q_d.ap()[ic])
                x3 = xall[:, 2244 * ic:2244 * (ic + 1)].rearrange(
                    "c (h w) -> c h w", h=34)
                first, last = (ic == 0), (ic == 15)
                for tap in range(9):
                    dy, dx = tap // 3, tap % 3
                    wq_t = wq[:, tap * 128:tap * 128 + 128]
                    for t in range(4):
                        win = x3[:, 8 * t + dy: 8 * t + dy + 8, dx: dx + 64]
                        nc.tensor.matmul(q1_ps[:, 512 * t: 512 * t + 512], wq_t, win,
                                         start=(first and tap == 0),
                                         stop=(last and tap == 8))

            q1_sb = opool.tile([128, 34 * 66], F16, name="q1_sb", tag="obig")
            q1o = q1_sb.rearrange("c (h w) -> c h w", h=34)
            q1v = q1_ps.rearrange("c (h w) -> c h w", h=32)
            _lrelu(nc, misc, q1v[:, 0:16, :], bcol(0), bcol(19), q1o[:, 1:17, 1:65], "q1e0")
            _lrelu(nc, misc, q1v[:, 16:32, :], bcol(0), bcol(19), q1o[:, 17:33, 1:65], "q1e1")
            nc.vector.tensor_copy(q1o[:, 0:1, 1:65], q1o[:, 2:3, 1:65])
            nc.vector.tensor_copy(q1o[:, 33:34, 1:65], q1o[:, 31:32, 1:65])
            nc.vector.tensor_copy(q1o[:, :, 0:1], q1o[:, :, 64:65])
            nc.vector.tensor_copy(q1o[:, :, 65:66], q1o[:, :, 1:2])

            # ============ k3/v3 convs (channel-sharded) + tiny AllGather =====
            k3_ps = ppool.tile([32, 65], F32, name="k3_ps", tag="pk")
            for jc in range(2):
                k2c3 = k2_sb[:, 105 * jc:105 * jc + 105].rearrange(
                    "c (h w) -> c h w", h=7)
                w3k = wpool.tile([128, 288], F16, name="w3k", tag="wB")
                nc.gpsimd.dma_start(w3k[:], w3k_d.ap()[jc])
                for tap in range(9):
                    dy, dx = tap // 3, tap % 3
                    wink = k2c3[:, dy: dy + 5, dx: dx + 13]
                    nc.tensor.matmul(k3_ps[:], w3k[:, tap * 32:tap * 32 + 32], wink,
                                     start=(jc == 0 and tap == 0), stop=(jc == 1 and tap == 8))
            v3_ps = ppool.tile([128, 65], F32, name="v3_ps", tag="pv")
            for ic in range(8):
                v2c3 = v2_sb[:, 105 * ic:105 * ic + 105].rearrange(
                    "c (h w) -> c h w", h=7)
                w3v = wpool.tile([128, 1152], F16, name="w3v", tag="wC")
                nc.gpsimd.dma_start(w3v[:], w3v_d.ap()[ic])
                for tap in range(9):
                    dy, dx = tap // 3, tap % 3
                    winv = v2c3[:, dy: dy + 5, dx: dx + 13]
                    nc.tensor.matmul(v3_ps[:], w3v[:, tap * 128:tap * 128 + 128], winv,
                                     start=(ic == 0 and tap == 0), stop=(ic == 7 and tap == 8))

            k3_sb = misc.tile([32, 65], F16, name="k3_sb")
            _lrelu(nc, misc, k3_ps[:], bcol(17)[0:32], bcol(36)[0:32], k3_sb[:], "k3e")
            v3_sb = misc.tile([128, 65], F16, name="v3_sb")
            _lrelu(nc, misc, v3_ps[:], bcol(18), bcol(37), v3_sb[:], "v3e")
            nc.sync.dma_start(agkv_in[0:2080].rearrange("(p f) -> p f", p=32), k3_sb[:])
            nc.sync.dma_start(agkv_in[2080:10400].rearrange("(p f) -> p f", p=128), v3_sb[:])
            nc.gpsimd.collective_compute("AllGather", ALU.bypass, replica_groups=RG,
                                         ins=[agkv_in.opt()], outs=[agkv_out.opt()])

            # ============ q2 partials -> fp16 AllToAll over position blocks ==
            w2q = opool.tile([128, 2304], F16, name="w2q", tag="wq2")
            nc.gpsimd.dma_start(w2q[:], w2q_d.ap())
            stg = opool.tile([128, 8 * 768], F16, name="stg", tag="stg")
            engs = [nc.scalar.copy,
                    lambda o, i: nc.vector.tensor_copy(o, i)]
            NE = len(engs)
            for cc in range(2):
                qp = ppool.tile([128, 2048], F32, name="qp", tag="pbig")
                for tap in range(9):
                    dy, dx = tap // 3, tap % 3
                    wslc = w2q[:, tap * 256 + 128 * cc: tap * 256 + 128 * cc + 128]
                    for t in range(4):
                        win = q1o[:, 8 * t + dy: 8 * t + dy + 8, dx: dx + 64]
                        nc.tensor.matmul(qp[:, 512 * t:512 * t + 512], wslc, win,
                                         start=(tap == 0), stop=(tap == 8))
                # stage position slices (rows 4j-1 .. 4j+5, reflect at edges)
                ei = 0
                for j in range(8):
                    base = 768 * j + 384 * cc
                    if j == 0:
                        engs[ei % NE](stg[:, base:base + 64], qp[:, 64:128]); ei += 1
                        engs[ei % NE](stg[:, base + 64:base + 384], qp[:, 0:320]); ei += 1
                    elif j == 7:
                        engs[ei % NE](stg[:, base:base + 320], qp[:, 1728:2048]); ei += 1
                        engs[ei % NE](stg[:, base + 320:base + 384], qp[:, 1920:1984]); ei += 1
                    else:
                        engs[ei % NE](stg[:, base:base + 384],
                                     qp[:, 256 * j - 64:256 * j + 320]); ei += 1
            for j in range(8):
                nc.sync.dma_start(a2a_in[j].rearrange("(c p f) -> p c f",
                                                      c=2, p=128),
                                  stg[:, 768 * j:768 * j + 768].rearrange(
                                      "p (c f) -> p c f", c=2))
            nc.gpsimd.collective_compute("AllToAll", ALU.bypass, replica_groups=RG,
                                         ins=[a2a_in.opt()], outs=[a2a_out.opt()])

            # v3 full -> transposed [65, 1024] (overlaps the AllToAll)
            ident = misc.tile([128, 128], F16)
            nc.sync.dma_start(ident[:], ident_d.ap())
            v3f = misc.tile([128, 520], F16, name="v3f")
            for i in range(8):
                nc.sync.dma_start(v3f[:, 65 * i:65 * i + 65],
                                  agkv_out[i, 2080:10400].rearrange(
                                      "(p f) -> p f", p=128))
            v3t = misc.tile([65, 1024], F16, name="v3t")
            for i in range(8):
                tps = ppool.tile([65, 128], F16, name="tps", tag="pk")
                nc.tensor.transpose(tps[:], v3f[:, 65 * i:65 * i + 65], ident[:])
                (nc.scalar.copy if i % 2 else nc.vector.tensor_copy)(
                    v3t[:, 128 * i:128 * i + 128], tps[:])
            k3f = misc.tile([128, 130], F16, name="k3f")
            for mm in range(2):
                for rr in range(4):
                    nc.sync.dma_start(
                        k3f[32 * rr:32 * rr + 32, 65 * mm:65 * mm + 65],
                        agkv_out[4 * mm + rr, 0:2080].rearrange(
                            "(p f) -> p f", p=32))

            # receive + local 8-way reduce of q2 partial slices
            recv = misc.tile([128, 6144], F16, name="recv")
            recv3 = recv.rearrange("p (s q) -> p s q", s=8)
            for c in range(2):
                nc.sync.dma_start(
                    recv3[:, :, 384 * c:384 * c + 384],
                    a2a_out[:, 49152 * c:49152 * c + 49152].rearrange(
                        "s (p f) -> p s f", p=128))
            r = lambda s: recv[:, 768 * s:768 * s + 768]
            tA = misc.tile([128, 768], F16, name="tA")
            tB = misc.tile([128, 768], F16, name="tB")
            tC = misc.tile([128, 768], F16, name="tC")
            tD = misc.tile([128, 768], F16, name="tD")
            nc.vector.tensor_tensor(tA[:], r(0), r(1), op=ALU.add)
            nc.gpsimd.tensor_tensor(tB[:], r(2), r(3), op=ALU.add)
            nc.vector.tensor_tensor(tC[:], r(4), r(5), op=ALU.add)
            nc.gpsimd.tensor_tensor(tD[:], r(6), r(7), op=ALU.add)
            nc.vector.tensor_tensor(tA[:], tA[:], tB[:], op=ALU.add)
            nc.gpsimd.tensor_tensor(tC[:], tC[:], tD[:], op=ALU.add)
            q2r = misc.tile([128, 768], F32, name="q2r")
            nc.vector.tensor_tensor(q2r[:], tA[:], tC[:], op=ALU.add)

            # q2 epilogue: bias+lrelu, wrap-pad W -> [128, 2 x (6*66)]
            q2_sb = opool.tile([128, 792], F16, name="q2_sb", tag="oq2")
            for cc in range(2):
                q2rg = q2r[:, 384 * cc:384 * cc + 384].rearrange(
                    "c (h w) -> c h w", h=6)
                q2o = q2_sb[:, 396 * cc:396 * cc + 396].rearrange(
                    "c (h w) -> c h w", h=6)
                _lrelu(nc, misc, q2rg[:], bcol(3 + cc), bcol(22 + cc),
                       q2o[:, :, 1:65], f"q2e{cc}")
                nc.vector.tensor_copy(q2o[:, :, 0:1], q2o[:, :, 64:65])
                nc.vector.tensor_copy(q2o[:, :, 65:66], q2o[:, :, 1:2])

            # ============ q3 conv (all 256 ch, own 4 rows) ===================
            w3qs = []
            for icc in range(2):
                w3q = opool.tile([128, 2304], F16, name="w3q", tag=f"wq3{icc}")
                nc.gpsimd.dma_start(w3q[:], w3q_d.ap()[icc])
                w3qs.append(w3q)
            q3_ps = ppool.tile([128, 512], F32, name="q3_ps", tag="pbig")
            for oc in range(2):
                for icc in range(2):
                    q2c3 = q2_sb[:, 396 * icc:396 * icc + 396].rearrange(
                        "c (h w) -> c h w", h=6)
                    for tap in range(9):
                        dy, dx = tap // 3, tap % 3
                        win = q2c3[:, dy: dy + 4, dx: dx + 64]
                        nc.tensor.matmul(q3_ps[:, 256 * oc:256 * oc + 256],
                                         w3qs[icc][:, tap * 256 + 128 * oc: tap * 256 + 128 * oc + 128],
                                         win, start=(icc == 0 and tap == 0),
                                         stop=(icc == 1 and tap == 8))
            q3_sb = misc.tile([128, 512], F16, name="q3_sb")
            for oc in range(2):
                _lrelu(nc, misc, q3_ps[:, 256 * oc:256 * oc + 256], bcol(15 + oc),
                       bcol(34 + oc), q3_sb[:, 256 * oc:256 * oc + 256], f"q3e{oc}")

            # ============ scores + softmax stats =============================
            sc_ps = ppool.tile([65, 256], F32, name="sc_ps", tag="pk")
            for icc in range(2):
                nc.tensor.matmul(sc_ps[:], k3f[:, 65 * icc:65 * icc + 65],
                                 q3_sb[:, 256 * icc:256 * icc + 256],
                                 start=(icc == 0), stop=(icc == 1))
            negmax = misc.tile([65, 1], F32)
            nc.vector.reduce_max(negmax[:], sc_ps[:], axis=AX.X, negate=True)
            esum = misc.tile([65, 1], F32)
            bexp = misc.tile([65, 256], F16, name="bexp")
            nc.scalar.activation(bexp[:], sc_ps[:], AF.Exp, bias=negmax[:, 0:1],
                                 accum_out=esum[:, 0:1])
            stats = misc.tile([65, 2], F32)
            nc.vector.tensor_scalar_mul(stats[:, 0:1], negmax[:], -1.0)
            nc.vector.tensor_copy(stats[:, 1:2], esum[:])
            nc.sync.dma_start(st_in[0:130].rearrange("(p c) -> p c", p=65), stats[:])
            nc.gpsimd.collective_compute("AllGather", ALU.bypass, replica_groups=RG,
                                         ins=[st_in.opt()], outs=[st_out.opt()])
            st8 = misc.tile([65, 16], F32)
            nc.sync.dma_start(st8.rearrange("p (r c) -> p r c", r=8),
                              st_out[:].rearrange("r (p c) -> p r c", p=65))
            mstar = misc.tile([65, 1], F32)
            nc.vector.reduce_max(mstar[:], st8[:, 0:16:2], axis=AX.X)
            texp = misc.tile([65, 8], F32)
            nc.vector.tensor_scalar_sub(texp[:], st8[:, 0:16:2], mstar[:, 0:1])
            nc.scalar.activation(texp[:], texp[:], AF.Exp)
            nc.vector.tensor_tensor(texp[:], texp[:], st8[:, 1:16:2], op=ALU.mult)
            dsum = misc.tile([65, 1], F32)
            nc.vector.reduce_sum(dsum[:], texp[:], axis=AX.X)
            rd = misc.tile([65, 1], F32)
            nc.vector.reciprocal(rd[:], dsum[:])
            g = misc.tile([65, 1], F32)
            nc.vector.tensor_tensor(g[:], stats[:, 0:1], mstar[:], op=ALU.subtract)
            nc.scalar.activation(g[:], g[:], AF.Exp)
            nc.vector.tensor_tensor(g[:], g[:], rd[:], op=ALU.mult)
            v3tg = misc.tile([65, 1024], F16, name="v3tg")
            nc.vector.tensor_scalar_mul(v3tg[:], v3t[:], g[:, 0:1])

            # ============ o + 1x1 projection (own 256 positions) =============
            oia = misc.tile([128, 2048], F16, name="oia")
            for i in range(8):
                ops = ppool.tile([128, 256], F32, name="ops", tag="pv")
                nc.tensor.matmul(ops[:], v3tg[:, 128 * i:128 * i + 128], bexp[:],
                                 start=True, stop=True)
                (nc.scalar.copy if i % 2 else nc.vector.tensor_copy)(
                    oia[:, 256 * i:256 * i + 256], ops[:])
            wpt = opool.tile([128, 8192], F16, name="wpt", tag="wpt")
            for i in range(8):
                nc.gpsimd.dma_start(wpt[:, 1024 * i:1024 * i + 1024], wp_d.ap()[i])
            for j in range(8):
                out_ps = ppool.tile([128, 256], F32, name="out_ps", tag="po")
                for icc in range(8):
                    nc.tensor.matmul(out_ps[:],
                                     wpt[:, 1024 * icc + 128 * j: 1024 * icc + 128 * j + 128],
                                     oia[:, 256 * icc:256 * icc + 256],
                                     start=(icc == 0), stop=(icc == 7))
                out_sb = misc.tile([128, 256], F32, name="out_sb", tag="osb", bufs=2)
                nc.vector.tensor_scalar_add(out_sb[:], out_ps[:], bcol(38 + j))
                nc.sync.dma_start(out_d.ap()[128 * j:128 * j + 128], out_sb[:])

    nc.compile()
    nc.m = get_hw_module(nc.m)
    return nc


def _prep_inputs(x, qw1, qb1, qw2, qb2, qw3, qb3, kw1, kb1, kw2, kb2, kw3, kb3,
                 vw1, vb1, vw2, vb2, vw3, vb3, pw, pb):
    f16 = np.float16
    x = np.ascontiguousarray(np.asarray(x).reshape(2048, 32, 64), dtype=np.float32)
    xp = np.concatenate([x[:, 1:2], x, x[:, 30:31]], axis=1)
    xp = np.concatenate([xp[:, :, -1:], xp, xp[:, :, :1]], axis=2)
    xpad = np.ascontiguousarray(xp.reshape(16, 128, 34 * 66), dtype=f16)

    def conv_w(wt, co_lo, co_n, nchunk):
        ws = np.asarray(wt)[co_lo:co_lo + co_n]           # (co_n, Ci, 3, 3)
        ci = ws.shape[1]
        a = ws.reshape(co_n, nchunk, ci // nchunk, 9)     # (co, ck, ci, tap)
        a = a.transpose(1, 2, 3, 0)                       # (ck, ci, tap, co)
        return np.ascontiguousarray(a.reshape(nchunk, ci // nchunk, 9 * co_n),
                                    dtype=f16)

    def conv_w_ci(wt, ci_lo):
        # full out-channels, my 128 input channels -> (128ci, 9*co)
        ws = np.asarray(wt)[:, ci_lo:ci_lo + 128]         # (co, 128, 3, 3)
        co = ws.shape[0]
        a = ws.reshape(co, 128, 9).transpose(1, 2, 0)     # (ci, tap, co)
        return np.ascontiguousarray(a.reshape(128, 9 * co), dtype=f16)

    # full q3 weights: (2 ic-chunk, 128 ci, 9 tap * 256 co)
    w3q_full = np.asarray(qw3).reshape(256, 2, 128, 9).transpose(1, 2, 3, 0)
    w3q_full = np.ascontiguousarray(w3q_full.reshape(2, 128, 2304), dtype=f16)

    in_maps = []
    for c in range(NCORES):
        m = {"xpad": xpad}
        m["w1q"] = conv_w(qw1, 128 * c, 128, 16)
        m["w1k"] = conv_w(kw1, 128 * c, 128, 16)
        m["w1v"] = conv_w(vw1, 128 * c, 128, 16)
        m["w2q"] = conv_w_ci(qw2, 128 * c)
        m["w2k"] = conv_w_ci(kw2, 128 * c)
        wv2 = np.asarray(vw2)[:, 128 * c:128 * c + 128]        # (1024co, 128ci, 3, 3)
        wv2 = wv2.reshape(2, 512, 128, 9).transpose(0, 2, 3, 1)  # (half, ci, tap, co512)
        m["w2v"] = np.ascontiguousarray(wv2.reshape(2, 128, 4608), dtype=f16)
        m["w3q"] = w3q_full
        m["w3k"] = conv_w(kw3, 32 * c, 32, 2)
        m["w3v"] = conv_w(vw3, 128 * c, 128, 8)
        m["wp"] = np.ascontiguousarray(
            np.asarray(pw)[:, :, 0, 0].T.reshape(8, 128, 1024), dtype=f16)
        bias = np.zeros((128, 46), np.float32)
        bias[:, 0] = qb1[128 * c:128 * c + 128]
        bias[:, 1] = kb1[128 * c:128 * c + 128]
        bias[:, 2] = vb1[128 * c:128 * c + 128]
        bias[:, 3] = qb2[0:128]
        bias[:, 4] = qb2[128:256]
        bias[:, 5] = kb2[0:128]
        bias[:, 6] = kb2[128:256]
        for i in range(8):
            bias[:, 7 + i] = vb2[128 * i:128 * i + 128]
        bias[:, 15] = qb3[0:128]
        bias[:, 16] = qb3[128:256]
        bias[0:32, 17] = kb3[32 * c:32 * c + 32]
        bias[:, 18] = vb3[128 * c:128 * c + 128]
        bias[:, 19:38] = 0.3 * bias[:, 0:19]
        for j in range(8):
            bias[:, 38 + j] = pb[128 * j:128 * j + 128]
        m["bias"] = bias
        in_maps.append(m)
    return in_maps


LAST_RESULT = None


def kernel(**inputs):
    global LAST_RESULT
    if "nc" not in _CACHE:
        _CACHE["nc"] = build_program()
    nc = _CACHE["nc"]
    in_maps = _prep_inputs(**{k: np.asarray(v) for k, v in inputs.items()})
    res = bass_utils.run_bass_kernel_spmd(nc, in_maps, core_ids=list(range(NCORES)))
    LAST_RESULT = res
    out = np.empty((1024, 32, 64), np.float32)
    for c in range(NCORES):
        out[:, 4 * c:4 * c + 4, :] = res.results[c]["out_shard"].reshape(1024, 4, 64)
    return np.ascontiguousarray(out.reshape(1, 1024, 32, 64))


# revision 23
# speedup vs baseline: 1.3991x; 1.0492x over previous
"""Trainium2 Bass kernel for nn_AttentionModel (sparse_attention), v2.

8-core distribution (fp16 matmul inputs, fp32 PSUM accumulation):
 - layer-1 convs channel-TP (128 out-ch/core); the k/v branch runs FIRST so
   its whole collective chain hides under the big q1 conv.
 - k1/v1 read stride-2 windows straight from the padded x (no separate
   decimated copy of x).
 - k2/v2: local partials over the core's 128 k1/v1 channels -> one fp16
   AllReduce (Shared out) -> replicated epilogues -> channel-sharded k3/v3
   convs -> tiny k3|v3 AllGather.
 - q2: local partials for ALL 256 channels -> fp16 AllToAll whose slices are
   position blocks (4 H-rows/core + reflected halo rows baked in at send);
   the receiver does a local 8-way add. Everything after (q3, scores,
   softmax, o, 1x1 proj) is local to the core's 256 positions.
 - softmax over the query axis is globalized with a tiny (max, expsum)
   stats AllGather.
"""
import sys
import numpy as np

for _p in ('/opt/trn_rl_repo',):
    if _p not in sys.path:
        sys.path.insert(0, _p)

import concourse.bass as bass
import concourse.bacc as bacc
import concourse.tile as tile
import concourse.mybir as mybir
from concourse import bass_utils
from concourse.bass_interp import get_hw_module

F32 = mybir.dt.float32
F16 = mybir.dt.float16
AF = mybir.ActivationFunctionType
ALU = mybir.AluOpType
AX = mybir.AxisListType

NCORES = 8
_CACHE = {}


def _lrelu(nc, sb, src_ap, bias_ap, bias3_ap, out_ap, name):
    """out = max(src + b, 0.3*src + 0.3b)  (LeakyReLU 0.3)."""
    P = src_ap.shape[0]
    free = int(np.prod(src_ap.shape[1:]))
    s = sb.tile([P, free], F32, name=f"{name}_s", tag="epi_s")
    t = sb.tile([P, free], F32, name=f"{name}_t", tag="epi_t")
    nc.scalar.activation(s[:], src_ap, AF.Identity, bias=bias_ap, scale=1.0)
    nc.scalar.activation(t[:], src_ap, AF.Identity, bias=bias3_ap, scale=0.3)
    nc.vector.tensor_tensor(out_ap, s[:], t[:], op=ALU.max)


def build_program():
    nc = bacc.Bacc("TRN2", target_bir_lowering=False, debug=False,
                   enable_asserts=True, num_devices=NCORES)

    xpad_d = nc.dram_tensor("xpad", [16, 128, 34 * 66], F16, kind="ExternalInput")
    w1q_d = nc.dram_tensor("w1q", [16, 128, 1152], F16, kind="ExternalInput")
    w1k_d = nc.dram_tensor("w1k", [16, 128, 1152], F16, kind="ExternalInput")
    w1v_d = nc.dram_tensor("w1v", [16, 128, 1152], F16, kind="ExternalInput")
    w2q_d = nc.dram_tensor("w2q", [128, 2304], F16, kind="ExternalInput")
    w2k_d = nc.dram_tensor("w2k", [128, 2304], F16, kind="ExternalInput")
    w2v_d = nc.dram_tensor("w2v", [2, 128, 4608], F16, kind="ExternalInput")
    w3q_d = nc.dram_tensor("w3q", [2, 128, 2304], F16, kind="ExternalInput")
    w3k_d = nc.dram_tensor("w3k", [2, 128, 288], F16, kind="ExternalInput")
    w3v_d = nc.dram_tensor("w3v", [8, 128, 1152], F16, kind="ExternalInput")
    wp_d = nc.dram_tensor("wp", [8, 128, 1024], F16, kind="ExternalInput")
    bias_d = nc.dram_tensor("bias", [128, 46], F32, kind="ExternalInput")
    out_d = nc.dram_tensor("out_shard", [1024, 256], F32, kind="ExternalOutput")
    RG = [list(range(NCORES))]

    with tile.TileContext(nc) as tc:
        with (
            tc.tile_pool(name="dram", bufs=1, space="DRAM") as dram,
            tc.tile_pool(name="xres", bufs=1) as xres,
            tc.tile_pool(name="wpool", bufs=2) as wpool,
            tc.tile_pool(name="opool", bufs=1) as opool,
            tc.tile_pool(name="ppool", bufs=1, space="PSUM") as ppool,
            tc.tile_pool(name="misc", bufs=1) as misc,
        ):
            # collective buffers
            ar_in = dram.tile([134400], F16)                 # k2|v2 partials
            ar_out = dram.tile([134400], F16, addr_space="Shared")
            agkv_in = dram.tile([10400], F16)                # k3 | v3 shards
            agkv_out = dram.tile([8, 10400], F16, addr_space="Shared")
            a2a_in0 = dram.tile([8, 49152], F16)             # q2 partial slices
            a2a_out0 = dram.tile([8, 49152], F16)
            a2a_in1 = dram.tile([8, 49152], F16)
            a2a_out1 = dram.tile([8, 49152], F16)
            st_in = dram.tile([130], F32)                    # softmax stats
            st_out = dram.tile([8, 130], F32, addr_space="Shared")

            biases = misc.tile([128, 46], F32)
            nc.sync.dma_start(biases[:], bias_d.ap())
            bcol = lambda j: biases[:, j:j + 1]

            # tiny warmup collective: pays the first-collective setup cost
            warm_in = dram.tile([128, 4], F32)
            warm_out = dram.tile([1024, 4], F32, addr_space="Shared")
            nc.sync.dma_start(warm_in[:], bias_d.ap()[:, 0:4])
            nc.gpsimd.collective_compute("AllGather", ALU.bypass, replica_groups=RG,
                                         ins=[warm_in.opt()], outs=[warm_out.opt()])

            # x resident in SBUF: 16 chunks of [128, 34*66] fp16, loaded
            # interleaved with the layer-1 weights so the first chunks land
            # fast (x on the SP DMA ring, weights on the Pool ring)
            xall = xres.tile([128, 16 * 2244], F16, name="xall")

            # ============ k1/v1 convs (stride-2 windows from xpad) ===========
            k1_ps = ppool.tile([128, 465], F32, name="k1_ps", tag="pk")
            v1_ps = ppool.tile([128, 465], F32, name="v1_ps", tag="pv")
            for ic in range(16):
                nc.sync.dma_start(xall[:, 2244 * ic:2244 * (ic + 1)],
                                  xpad_d.ap()[ic])
                wk = wpool.tile([128, 1152], F16, name="wk", tag="wB")
                nc.gpsimd.dma_start(wk[:], w1k_d.ap()[ic])
                wv = wpool.tile([128, 1152], F16, name="wv", tag="wC")
                nc.gpsimd.dma_start(wv[:], w1v_d.ap()[ic])
                x3 = xall[:, 2244 * ic:2244 * (ic + 1)].rearrange(
                    "c (h w) -> c h w", h=34)
                first, last = (ic == 0), (ic == 15)
                for tap in range(9):
                    dy, dx = tap // 3, tap % 3
                    win = x3[:, 1 + dy: 1 + dy + 29: 2, 1 + dx: 1 + dx + 61: 2]
                    nc.tensor.matmul(k1_ps[:], wk[:, tap * 128:tap * 128 + 128], win,
                                     start=(first and tap == 0), stop=(last and tap == 8))
                    nc.tensor.matmul(v1_ps[:], wv[:, tap * 128:tap * 128 + 128], win,
                                     start=(first and tap == 0), stop=(last and tap == 8))

            k1_sb = opool.tile([128, 465], F16, name="k1_sb", tag="okv")
            _lrelu(nc, misc, k1_ps[:], bcol(1), bcol(20), k1_sb[:], "k1e")
            v1_sb = opool.tile([128, 465], F16, name="v1_sb", tag="ovv")
            _lrelu(nc, misc, v1_ps[:], bcol(2), bcol(21), v1_sb[:], "v1e")
            k1o = k1_sb.rearrange("c (h w) -> c h w", h=15)
            v1o = v1_sb.rearrange("c (h w) -> c h w", h=15)

            # ============ k2/v2 partials + fp16 AllReduce ====================
            w2k = opool.tile([128, 2304], F16, name="w2k", tag="wk2")
            nc.gpsimd.dma_start(w2k[:], w2k_d.ap())
            kp = ppool.tile([128, 210], F32, name="kp", tag="pk")
            for cc in range(2):
                for tap in range(9):
                    dy, dx = tap // 3, tap % 3
                    wink = k1o[:, dy: dy + 13: 2, dx: dx + 29: 2]
                    nc.tensor.matmul(kp[:, 105 * cc:105 * cc + 105],
                                     w2k[:, tap * 256 + 128 * cc: tap * 256 + 128 * cc + 128],
                                     wink, start=(tap == 0), stop=(tap == 8))
            vp = ppool.tile([128, 1024], F32, name="vp", tag="pbig")
            for vh in range(2):
                w2v = wpool.tile([128, 4608], F16, name="w2v", tag="wv2")
                nc.gpsimd.dma_start(w2v[:], w2v_d.ap()[vh])
                for c4 in range(4):
                    cc = 4 * vh + c4
                    for tap in range(9):
                        dy, dx = tap // 3, tap % 3
                        winv = v1o[:, dy: dy + 13: 2, dx: dx + 29: 2]
                        nc.tensor.matmul(vp[:, 128 * cc:128 * cc + 105],
                                         w2v[:, tap * 512 + 128 * c4: tap * 512 + 128 * c4 + 128],
                                         winv, start=(tap == 0), stop=(tap == 8))
            kps = misc.tile([128, 210], F16, name="kps", tag="rsk")
            nc.scalar.copy(kps[:], kp[:])
            vps = misc.tile([128, 840], F16, name="vps", tag="rsv")
            nc.vector.tensor_copy(vps.rearrange("p (c f) -> p c f", c=8),
                                  vp.rearrange("p (c f) -> p c f", c=8)[:, :, 0:105])
            nc.sync.dma_start(ar_in[0:26880].rearrange("(c p f) -> p c f", c=2, p=128),
                              kps.rearrange("p (c f) -> p c f", c=2))
            nc.sync.dma_start(ar_in[26880:134400].rearrange("(c p f) -> p c f", c=8, p=128),
                              vps.rearrange("p (c f) -> p c f", c=8))
            nc.gpsimd.collective_compute("AllReduce", ALU.add, replica_groups=RG,
                                         ins=[ar_in.opt()], outs=[ar_out.opt()])

            # k2/v2 epilogues on the reduced FULL tensors (overlap q1)
            k2r = misc.tile([128, 210], F16, name="k2r", tag="rsk2")
            nc.sync.dma_start(k2r.rearrange("p (c f) -> p c f", c=2),
                              ar_out[0:26880].rearrange("(c p f) -> p c f", c=2, p=128))
            v2r = misc.tile([128, 840], F16, name="v2r", tag="rsv2")
            nc.sync.dma_start(v2r.rearrange("p (c f) -> p c f", c=8),
                              ar_out[26880:134400].rearrange("(c p f) -> p c f", c=8, p=128))
            k2_sb = opool.tile([128, 210], F16, name="k2_sb", tag="okv2")
            for cc in range(2):
                _lrelu(nc, misc, k2r[:, 105 * cc:105 * cc + 105], bcol(5 + cc),
                       bcol(24 + cc), k2_sb[:, 105 * cc:105 * cc + 105], f"k2e{cc}")
            v2_sb = opool.tile([128, 840], F16, name="v2_sb", tag="ovv2")
            for cc in range(8):
                _lrelu(nc, misc, v2r[:, 105 * cc:105 * cc + 105], bcol(7 + cc),
                       bcol(26 + cc), v2_sb[:, 105 * cc:105 * cc + 105], f"v2e{cc}")

            # ============ q1 conv ============================================
            q1_ps = ppool.tile([128, 2048], F32, name="q1_ps", tag="pbig")
            for ic in range(16):
                wq = wpool.tile([128, 1152], F16, name="wq", tag="wA")
                nc.gpsimd.dma_start(wq[:], w1q_d.ap()[ic])
                x3 = xall[:, 2244 * ic:2244 * (ic + 1)].rearrange(
                    "c (h w) -> c h w", h=34)
                first, last = (ic == 0), (ic == 15)
                for tap in range(9):
                    dy, dx = tap // 3, tap % 3
                    wq_t = wq[:, tap * 128:tap * 128 + 128]
                    for t in range(4):
                        win = x3[:, 8 * t + dy: 8 * t + dy + 8, dx: dx + 64]
                        nc.tensor.matmul(q1_ps[:, 512 * t: 512 * t + 512], wq_t, win,
                                         start=(first and tap == 0),
                                         stop=(last and tap == 8))

            q1_sb = opool.tile([128, 34 * 66], F16, name="q1_sb", tag="obig")
            q1o = q1_sb.rearrange("c (h w) -> c h w", h=34)
            q1v = q1_ps.rearrange("c (h w) -> c h w", h=32)
            _lrelu(nc, misc, q1v[:, 0:16, :], bcol(0), bcol(19), q1o[:, 1:17, 1:65], "q1e0")
            _lrelu(nc, misc, q1v[:, 16:32, :], bcol(0), bcol(19), q1o[:, 17:33, 1:65], "q1e1")
            nc.vector.tensor_copy(q1o[:, 0:1, 1:65], q1o[:, 2:3, 1:65])
            nc.vector.tensor_copy(q1o[:, 33:34, 1:65], q1o[:, 31:32, 1:65])
            nc.vector.tensor_copy(q1o[:, :, 0:1], q1o[:, :, 64:65])
            nc.vector.tensor_copy(q1o[:, :, 65:66], q1o[:, :, 1:2])

            # ============ k3/v3 convs (channel-sharded) + tiny AllGather =====
            k3_ps = ppool.tile([32, 65], F32, name="k3_ps", tag="pk")
            for jc in range(2):
                k2c3 = k2_sb[:, 105 * jc:105 * jc + 105].rearrange(
                    "c (h w) -> c h w", h=7)
                w3k = wpool.tile([128, 288], F16, name="w3k", tag="wB")
                nc.gpsimd.dma_start(w3k[:], w3k_d.ap()[jc])
                for tap in range(9):
                    dy, dx = tap // 3, tap % 3
                    wink = k2c3[:, dy: dy + 5, dx: dx + 13]
                    nc.tensor.matmul(k3_ps[:], w3k[:, tap * 32:tap * 32 + 32], wink,
                                     start=(jc == 0 and tap == 0), stop=(jc == 1 and tap == 8))
            v3_ps = ppool.tile([128, 65], F32, name="v3_ps", tag="pv")
            for ic in range(8):
                v2c3 = v2_sb[:, 105 * ic:105 * ic + 105].rearrange(
                    "c (h w) -> c h w", h=7)
                w3v = wpool.tile([128, 1152], F16, name="w3v", tag="wC")
                nc.gpsimd.dma_start(w3v[:], w3v_d.ap()[ic])
                for tap in range(9):
                    dy, dx = tap // 3, tap % 3
                    winv = v2c3[:, dy: dy + 5, dx: dx + 13]
                    nc.tensor.matmul(v3_ps[:], w3v[:, tap * 128:tap * 128 + 128], winv,
                                     start=(ic == 0 and tap == 0), stop=(ic == 7 and tap == 8))

            k3_sb = misc.tile([32, 65], F16, name="k3_sb")
            _lrelu(nc, misc, k3_ps[:], bcol(17)[0:32], bcol(36)[0:32], k3_sb[:], "k3e")
            v3_sb = misc.tile([128, 65], F16, name="v3_sb")
            _lrelu(nc, misc, v3_ps[:], bcol(18), bcol(37), v3_sb[:], "v3e")
            nc.sync.dma_start(agkv_in[0:2080].rearrange("(p f) -> p f", p=32), k3_sb[:])
            nc.sync.dma_start(agkv_in[2080:10400].rearrange("(p f) -> p f", p=128), v3_sb[:])
            nc.gpsimd.collective_compute("AllGather", ALU.bypass, replica_groups=RG,
                                         ins=[agkv_in.opt()], outs=[agkv_out.opt()])

            # ============ q2 partials -> 2x fp16 AllToAll over position blocks
            w2q = opool.tile([128, 2304], F16, name="w2q", tag="wq2")
            nc.gpsimd.dma_start(w2q[:], w2q_d.ap())
            wpt = opool.tile([128, 8192], F16, name="wpt", tag="wpt")
            for i in range(8):
                nc.gpsimd.dma_start(wpt[:, 1024 * i:1024 * i + 1024], wp_d.ap()[i])
            w3qs = []
            for icc in range(2):
                w3q = opool.tile([128, 2304], F16, name="w3q", tag=f"wq3{icc}")
                nc.gpsimd.dma_start(w3q[:], w3q_d.ap()[icc])
                w3qs.append(w3q)

            engs = [nc.scalar.copy,
                    lambda o, i: nc.vector.tensor_copy(o, i)]
            NE = len(engs)
            a2a_ins = [a2a_in0, a2a_in1]
            a2a_outs = [a2a_out0, a2a_out1]
            for cc in range(2):
                qp = ppool.tile([128, 2048], F32, name="qp", tag="pbig")
                for tap in range(9):
                    dy, dx = tap // 3, tap % 3
                    wslc = w2q[:, tap * 256 + 128 * cc: tap * 256 + 128 * cc + 128]
                    for t in range(4):
                        win = q1o[:, 8 * t + dy: 8 * t + dy + 8, dx: dx + 64]
                        nc.tensor.matmul(qp[:, 512 * t:512 * t + 512], wslc, win,
                                         start=(tap == 0), stop=(tap == 8))
                # stage position slices (rows 4j-1 .. 4j+5, reflect at edges)
                stg = opool.tile([128, 3072], F16, name=f"stg{cc}", tag=f"stg{cc}")
                ei = 0
                for j in range(8):
                    base = 384 * j
                    if j == 0:
                        engs[ei % NE](stg[:, base:base + 64], qp[:, 64:128]); ei += 1
                        engs[ei % NE](stg[:, base + 64:base + 384], qp[:, 0:320]); ei += 1
                    elif j == 7:
                        engs[ei % NE](stg[:, base:base + 320], qp[:, 1728:2048]); ei += 1
                        engs[ei % NE](stg[:, base + 320:base + 384], qp[:, 1920:1984]); ei += 1
                    else:
                        engs[ei % NE](stg[:, base:base + 384],
                                      qp[:, 256 * j - 64:256 * j + 320]); ei += 1
                for j in range(8):
                    nc.sync.dma_start(
                        a2a_ins[cc][j].rearrange("(p f) -> p f", p=128),
                        stg[:, 384 * j:384 * j + 384])
                nc.gpsimd.collective_compute("AllToAll", ALU.bypass, replica_groups=RG,
                                             ins=[a2a_ins[cc].opt()],
                                             outs=[a2a_outs[cc].opt()])

            # v3/k3 full (from the kv AllGather) + Yt = v3^T wp, all overlapping
            # the AllToAlls. Yt folds the 1x1 projection through the attention
            # values so the post-softmax work is just one [65]-contraction.
            v3f = misc.tile([128, 520], F16, name="v3f", tag="v3f")
            for i in range(8):
                nc.sync.dma_start(v3f[:, 65 * i:65 * i + 65],
                                  agkv_out[i, 2080:10400].rearrange(
                                      "(p f) -> p f", p=128))
            k3f = misc.tile([128, 130], F16, name="k3f", tag="k3f")
            for mm in range(2):
                for rr in range(4):
                    nc.sync.dma_start(
                        k3f[32 * rr:32 * rr + 32, 65 * mm:65 * mm + 65],
                        agkv_out[4 * mm + rr, 0:2080].rearrange(
                            "(p f) -> p f", p=32))
            yt_ps = ppool.tile([65, 1024], F32, name="yt_ps", tag="pyt")
            for i in range(8):
                for h in range(2):
                    nc.tensor.matmul(yt_ps[:, 512 * h:512 * h + 512],
                                     v3f[:, 65 * i:65 * i + 65],
                                     wpt[:, 1024 * i + 512 * h:1024 * i + 512 * h + 512],
                                     start=(i == 0), stop=(i == 7))

            # receive + local 8-way reduce + epilogue, per cc chunk
            q2_sb = opool.tile([128, 792], F16, name="q2_sb", tag="oq2")
            for c in range(2):
                recvc = misc.tile([128, 3072], F16, name=f"recv{c}", tag=f"recv{c}")
                nc.sync.dma_start(recvc.rearrange("p (s f) -> p s f", s=8),
                                  a2a_outs[c][:].rearrange("s (p f) -> p s f", p=128))
                r = lambda s: recvc[:, 384 * s:384 * s + 384]
                tA = misc.tile([128, 384], F16, name=f"tA{c}", tag=f"tA{c}")
                tB = misc.tile([128, 384], F16, name=f"tB{c}", tag=f"tB{c}")
                tC = misc.tile([128, 384], F16, name=f"tC{c}", tag=f"tC{c}")
                tD = misc.tile([128, 384], F16, name=f"tD{c}", tag=f"tD{c}")
                nc.vector.tensor_tensor(tA[:], r(0), r(1), op=ALU.add)
                nc.gpsimd.tensor_tensor(tB[:], r(2), r(3), op=ALU.add)
                nc.vector.tensor_tensor(tC[:], r(4), r(5), op=ALU.add)
                nc.gpsimd.tensor_tensor(tD[:], r(6), r(7), op=ALU.add)
                nc.vector.tensor_tensor(tA[:], tA[:], tB[:], op=ALU.add)
                nc.gpsimd.tensor_tensor(tC[:], tC[:], tD[:], op=ALU.add)
                q2rc = misc.tile([128, 384], F32, name=f"q2r{c}", tag=f"q2r{c}")
                nc.vector.tensor_tensor(q2rc[:], tA[:], tC[:], op=ALU.add)
                q2rg = q2rc.rearrange("c (h w) -> c h w", h=6)
                q2o = q2_sb[:, 396 * c:396 * c + 396].rearrange(
                    "c (h w) -> c h w", h=6)
                _lrelu(nc, misc, q2rg[:], bcol(3 + c), bcol(22 + c),
                       q2o[:, :, 1:65], f"q2e{c}")
                nc.vector.tensor_copy(q2o[:, :, 0:1], q2o[:, :, 64:65])
                nc.vector.tensor_copy(q2o[:, :, 65:66], q2o[:, :, 1:2])

            # ============ q3 conv (all 256 ch, own 4 rows) ===================
            q3_ps = ppool.tile([128, 1024], F32, name="q3_ps", tag="pbig")
            for icc in range(2):
                q2c3 = q2_sb[:, 396 * icc:396 * icc + 396].rearrange(
                    "c (h w) -> c h w", h=6)
                for oc in range(2):
                    for tap in range(9):
                        dy, dx = tap // 3, tap % 3
                        win = q2c3[:, dy: dy + 4, dx: dx + 64]
                        nc.tensor.matmul(q3_ps[:, 512 * oc:512 * oc + 256],
                                         w3qs[icc][:, tap * 256 + 128 * oc: tap * 256 + 128 * oc + 128],
                                         win, start=(icc == 0 and tap == 0),
                                         stop=(icc == 1 and tap == 8))
            q3_sb = misc.tile([128, 512], F16, name="q3_sb", tag="q3sb")
            for oc in range(2):
                _lrelu(nc, misc, q3_ps[:, 512 * oc:512 * oc + 256], bcol(15 + oc),
                       bcol(34 + oc), q3_sb[:, 256 * oc:256 * oc + 256], f"q3e{oc}")

            # ============ scores + softmax stats =============================
            sc_ps = ppool.tile([65, 256], F32, name="sc_ps", tag="pk")
            for icc in range(2):
                nc.tensor.matmul(sc_ps[:], k3f[:, 65 * icc:65 * icc + 65],
                                 q3_sb[:, 256 * icc:256 * icc + 256],
                                 start=(icc == 0), stop=(icc == 1))
            negmax = misc.tile([65, 1], F32, name="negmax", tag="negmax")
            nc.vector.reduce_max(negmax[:], sc_ps[:], axis=AX.X, negate=True)
            esum = misc.tile([65, 1], F32, name="esum", tag="esum")
            bexp = misc.tile([65, 256], F16, name="bexp", tag="bexp")
            nc.scalar.activation(bexp[:], sc_ps[:], AF.Exp, bias=negmax[:, 0:1],
                                 accum_out=esum[:, 0:1])
            stats = misc.tile([65, 2], F32, name="stats", tag="stats")
            nc.vector.tensor_scalar_mul(stats[:, 0:1], negmax[:], -1.0)
            nc.vector.tensor_copy(stats[:, 1:2], esum[:])
            nc.sync.dma_start(st_in[0:130].rearrange("(p c) -> p c", p=65), stats[:])
            nc.gpsimd.collective_compute("AllGather", ALU.bypass, replica_groups=RG,
                                         ins=[st_in.opt()], outs=[st_out.opt()])
            st8 = misc.tile([65, 16], F32, name="st8", tag="st8")
            nc.sync.dma_start(st8.rearrange("p (r c) -> p r c", r=8),
                              st_out[:].rearrange("r (p c) -> p r c", p=65))
            mstar = misc.tile([65, 1], F32, name="mstar", tag="mstar")
            nc.vector.reduce_max(mstar[:], st8[:, 0:16:2], axis=AX.X)
            texp = misc.tile([65, 8], F32, name="texp", tag="texp")
            nc.vector.tensor_scalar_sub(texp[:], st8[:, 0:16:2], mstar[:, 0:1])
            nc.scalar.activation(texp[:], texp[:], AF.Exp)
            nc.vector.tensor_tensor(texp[:], texp[:], st8[:, 1:16:2], op=ALU.mult)
            dsum = misc.tile([65, 1], F32, name="dsum", tag="dsum")
            nc.vector.reduce_sum(dsum[:], texp[:], axis=AX.X)
            rd = misc.tile([65, 1], F32, name="rd", tag="rd")
            nc.vector.reciprocal(rd[:], dsum[:])
            g = misc.tile([65, 1], F32, name="g", tag="g")
            nc.vector.tensor_tensor(g[:], stats[:, 0:1], mstar[:], op=ALU.subtract)
            nc.scalar.activation(g[:], g[:], AF.Exp)
            nc.vector.tensor_tensor(g[:], g[:], rd[:], op=ALU.mult)

            # ============ output: (g * Yt)^T [65] contraction with bexp ======
            ytg = misc.tile([65, 1024], F16, name="ytg", tag="ytg")
            nc.scalar.activation(ytg[:], yt_ps[:], AF.Identity, scale=g[:, 0:1])
            for j in range(8):
                out_ps = ppool.tile([128, 256], F32, name="out_ps",
                                    tag=("pv" if j % 2 else "pk"))
                nc.tensor.matmul(out_ps[:], ytg[:, 128 * j:128 * j + 128], bexp[:],
                                 start=True, stop=True)
                out_sb = misc.tile([128, 256], F32, name="out_sb", tag="osb", bufs=2)
                nc.vector.tensor_scalar_add(out_sb[:], out_ps[:], bcol(38 + j))
                nc.sync.dma_start(out_d.ap()[128 * j:128 * j + 128], out_sb[:])

    nc.compile()
    nc.m = get_hw_module(nc.m)
    return nc


def _prep_inputs(x, qw1, qb1, qw2, qb2, qw3, qb3, kw1, kb1, kw2, kb2, kw3, kb3,
                 vw1, vb1, vw2, vb2, vw3, vb3, pw, pb):
    f16 = np.float16
    x = np.ascontiguousarray(np.asarray(x).reshape(2048, 32, 64), dtype=np.float32)
    xp = np.concatenate([x[:, 1:2], x, x[:, 30:31]], axis=1)
    xp = np.concatenate([xp[:, :, -1:], xp, xp[:, :, :1]], axis=2)
    xpad = np.ascontiguousarray(xp.reshape(16, 128, 34 * 66), dtype=f16)

    def conv_w(wt, co_lo, co_n, nchunk):
        ws = np.asarray(wt)[co_lo:co_lo + co_n]           # (co_n, Ci, 3, 3)
        ci = ws.shape[1]
        a = ws.reshape(co_n, nchunk, ci // nchunk, 9)     # (co, ck, ci, tap)
        a = a.transpose(1, 2, 3, 0)                       # (ck, ci, tap, co)
        return np.ascontiguousarray(a.reshape(nchunk, ci // nchunk, 9 * co_n),
                                    dtype=f16)

    def conv_w_ci(wt, ci_lo):
        # full out-channels, my 128 input channels -> (128ci, 9*co)
        ws = np.asarray(wt)[:, ci_lo:ci_lo + 128]         # (co, 128, 3, 3)
        co = ws.shape[0]
        a = ws.reshape(co, 128, 9).transpose(1, 2, 0)     # (ci, tap, co)
        return np.ascontiguousarray(a.reshape(128, 9 * co), dtype=f16)

    # full q3 weights: (2 ic-chunk, 128 ci, 9 tap * 256 co)
    w3q_full = np.asarray(qw3).reshape(256, 2, 128, 9).transpose(1, 2, 3, 0)
    w3q_full = np.ascontiguousarray(w3q_full.reshape(2, 128, 2304), dtype=f16)

    in_maps = []
    for c in range(NCORES):
        m = {"xpad": xpad}
        m["w1q"] = conv_w(qw1, 128 * c, 128, 16)
        m["w1k"] = conv_w(kw1, 128 * c, 128, 16)
        m["w1v"] = conv_w(vw1, 128 * c, 128, 16)
        m["w2q"] = conv_w_ci(qw2, 128 * c)
        m["w2k"] = conv_w_ci(kw2, 128 * c)
        wv2 = np.asarray(vw2)[:, 128 * c:128 * c + 128]        # (1024co, 128ci, 3, 3)
        wv2 = wv2.reshape(2, 512, 128, 9).transpose(0, 2, 3, 1)  # (half, ci, tap, co512)
        m["w2v"] = np.ascontiguousarray(wv2.reshape(2, 128, 4608), dtype=f16)
        m["w3q"] = w3q_full
        m["w3k"] = conv_w(kw3, 32 * c, 32, 2)
        m["w3v"] = conv_w(vw3, 128 * c, 128, 8)
        m["wp"] = np.ascontiguousarray(
            np.asarray(pw)[:, :, 0, 0].T.reshape(8, 128, 1024), dtype=f16)
        bias = np.zeros((128, 46), np.float32)
        bias[:, 0] = qb1[128 * c:128 * c + 128]
        bias[:, 1] = kb1[128 * c:128 * c + 128]
        bias[:, 2] = vb1[128 * c:128 * c + 128]
        bias[:, 3] = qb2[0:128]
        bias[:, 4] = qb2[128:256]
        bias[:, 5] = kb2[0:128]
        bias[:, 6] = kb2[128:256]
        for i in range(8):
            bias[:, 7 + i] = vb2[128 * i:128 * i + 128]
        bias[:, 15] = qb3[0:128]
        bias[:, 16] = qb3[128:256]
        bias[0:32, 17] = kb3[32 * c:32 * c + 32]
        bias[:, 18] = vb3[128 * c:128 * c + 128]
        bias[:, 19:38] = 0.3 * bias[:, 0:19]
        for j in range(8):
            bias[:, 38 + j] = pb[128 * j:128 * j + 128]
        m["bias"] = bias
        in_maps.append(m)
    return in_maps


LAST_RESULT = None


def kernel(**inputs):
    global LAST_RESULT
    if "nc" not in _CACHE:
        _CACHE["nc"] = build_program()
    nc = _CACHE["nc"]
    in_maps = _prep_inputs(**{k: np.asarray(v) for k, v in inputs.items()})
    res = bass_utils.run_bass_kernel_spmd(nc, in_maps, core_ids=list(range(NCORES)))
    LAST_RESULT = res
    out = np.empty((1024, 32, 64), np.float32)
    for c in range(NCORES):
        out[:, 4 * c:4 * c + 4, :] = res.results[c]["out_shard"].reshape(1024, 4, 64)
    return np.ascontiguousarray(out.reshape(1, 1024, 32, 64))


# revision 33
# speedup vs baseline: 1.4773x; 1.0559x over previous
"""Trainium2 Bass kernel for nn_AttentionModel (sparse_attention), v2.

8-core distribution (fp16 matmul inputs, fp32 PSUM accumulation):
 - layer-1 convs channel-TP (128 out-ch/core); the k/v branch runs FIRST so
   its whole collective chain hides under the big q1 conv.
 - k1/v1 read stride-2 windows straight from the padded x (no separate
   decimated copy of x).
 - k2/v2: local partials over the core's 128 k1/v1 channels -> one fp16
   AllReduce (Shared out) -> replicated epilogues -> channel-sharded k3/v3
   convs -> tiny k3|v3 AllGather.
 - q2: local partials for ALL 256 channels -> fp16 AllToAll whose slices are
   position blocks (4 H-rows/core + reflected halo rows baked in at send);
   the receiver does a local 8-way add. Everything after (q3, scores,
   softmax, o, 1x1 proj) is local to the core's 256 positions.
 - softmax over the query axis is globalized with a tiny (max, expsum)
   stats AllGather.
"""
import sys
import numpy as np

for _p in ('/opt/trn_rl_repo',):
    if _p not in sys.path:
        sys.path.insert(0, _p)

import concourse.bass as bass
import concourse.bacc as bacc
import concourse.tile as tile
import concourse.mybir as mybir
from concourse import bass_utils
from concourse.bass_interp import get_hw_module

F32 = mybir.dt.float32
F16 = mybir.dt.float16
AF = mybir.ActivationFunctionType
ALU = mybir.AluOpType
AX = mybir.AxisListType

NCORES = 8
_CACHE = {}


def _lrelu(nc, sb, src_ap, bias_ap, bias3_ap, out_ap, name):
    """out = max(src + b, 0.3*src + 0.3b)  (LeakyReLU 0.3)."""
    P = src_ap.shape[0]
    free = int(np.prod(src_ap.shape[1:]))
    s = sb.tile([P, free], F32, name=f"{name}_s", tag="epi_s")
    t = sb.tile([P, free], F32, name=f"{name}_t", tag="epi_t")
    nc.scalar.activation(s[:], src_ap, AF.Identity, bias=bias_ap, scale=1.0)
    nc.scalar.activation(t[:], src_ap, AF.Identity, bias=bias3_ap, scale=0.3)
    nc.vector.tensor_tensor(out_ap, s[:], t[:], op=ALU.max)


def build_program():
    nc = bacc.Bacc("TRN2", target_bir_lowering=False, debug=False,
                   enable_asserts=True, num_devices=NCORES)

    xpad_d = nc.dram_tensor("xpad", [16, 128, 34 * 66], F16, kind="ExternalInput")
    w1q_d = nc.dram_tensor("w1q", [16, 128, 1152], F16, kind="ExternalInput")
    w1k_d = nc.dram_tensor("w1k", [16, 128, 1152], F16, kind="ExternalInput")
    w1v_d = nc.dram_tensor("w1v", [16, 128, 1152], F16, kind="ExternalInput")
    w2q_d = nc.dram_tensor("w2q", [128, 2304], F16, kind="ExternalInput")
    w2k_d = nc.dram_tensor("w2k", [128, 2304], F16, kind="ExternalInput")
    w2v_d = nc.dram_tensor("w2v", [2, 128, 4608], F16, kind="ExternalInput")
    w3q_d = nc.dram_tensor("w3q", [2, 128, 2304], F16, kind="ExternalInput")
    w3k_d = nc.dram_tensor("w3k", [2, 128, 288], F16, kind="ExternalInput")
    w3v_d = nc.dram_tensor("w3v", [8, 128, 1152], F16, kind="ExternalInput")
    wp_d = nc.dram_tensor("wp", [8, 128, 1024], F16, kind="ExternalInput")
    bias_d = nc.dram_tensor("bias", [128, 46], F32, kind="ExternalInput")
    out_d = nc.dram_tensor("out_shard", [1024, 256], F16, kind="ExternalOutput")
    RG = [list(range(NCORES))]

    with tile.TileContext(nc) as tc:
        with (
            tc.tile_pool(name="dram", bufs=1, space="DRAM") as dram,
            tc.tile_pool(name="xres", bufs=1) as xres,
            tc.tile_pool(name="wpool", bufs=2) as wpool,
            tc.tile_pool(name="opool", bufs=1) as opool,
            tc.tile_pool(name="ppool", bufs=1, space="PSUM") as ppool,
            tc.tile_pool(name="misc", bufs=1) as misc,
        ):
            # collective buffers
            ar_in = dram.tile([134400], F16)                 # k2|v2 partials
            ar_out = dram.tile([134400], F16, addr_space="Shared")
            agkv_in = dram.tile([10400], F16)                # k3 | v3 shards
            agkv_out = dram.tile([8, 10400], F16, addr_space="Shared")
            a2a_in = dram.tile([8, 98304], F16)              # q2 partial slices
            a2a_out = dram.tile([8, 98304], F16)
            st_in = dram.tile([130], F32)                    # softmax stats
            st_out = dram.tile([8, 130], F32, addr_space="Shared")

            biases = misc.tile([128, 46], F32)
            nc.sync.dma_start(biases[:], bias_d.ap())
            bcol = lambda j: biases[:, j:j + 1]

            # tiny warmup collective: pays the first-collective setup cost
            warm_in = dram.tile([128, 4], F32)
            warm_out = dram.tile([1024, 4], F32, addr_space="Shared")
            nc.sync.dma_start(warm_in[:], bias_d.ap()[:, 0:4])
            nc.gpsimd.collective_compute("AllGather", ALU.bypass, replica_groups=RG,
                                         ins=[warm_in.opt()], outs=[warm_out.opt()])

            # x resident in SBUF: 16 chunks of [128, 34*66] fp16, loaded
            # interleaved with the layer-1 weights so the first chunks land
            # fast (x on the SP DMA ring, weights on the Pool ring)
            xall = xres.tile([128, 16 * 2244], F16, name="xall")

            # ============ k1/v1 convs (stride-2 windows from xpad) ===========
            k1_ps = ppool.tile([128, 465], F32, name="k1_ps", tag="pk")
            v1_ps = ppool.tile([128, 465], F32, name="v1_ps", tag="pv")
            for ic in range(16):
                nc.sync.dma_start(xall[:, 2244 * ic:2244 * (ic + 1)],
                                  xpad_d.ap()[ic])
                wk = wpool.tile([128, 1152], F16, name="wk", tag="wB")
                nc.gpsimd.dma_start(wk[:], w1k_d.ap()[ic])
                wv = wpool.tile([128, 1152], F16, name="wv", tag="wC")
                nc.gpsimd.dma_start(wv[:], w1v_d.ap()[ic])
                x3 = xall[:, 2244 * ic:2244 * (ic + 1)].rearrange(
                    "c (h w) -> c h w", h=34)
                first, last = (ic == 0), (ic == 15)
                for tap in range(9):
                    dy, dx = tap // 3, tap % 3
                    win = x3[:, 1 + dy: 1 + dy + 29: 2, 1 + dx: 1 + dx + 61: 2]
                    nc.tensor.matmul(k1_ps[:], wk[:, tap * 128:tap * 128 + 128], win,
                                     start=(first and tap == 0), stop=(last and tap == 8))
                    nc.tensor.matmul(v1_ps[:], wv[:, tap * 128:tap * 128 + 128], win,
                                     start=(first and tap == 0), stop=(last and tap == 8))

            k1_sb = opool.tile([128, 465], F16, name="k1_sb", tag="okv")
            _lrelu(nc, misc, k1_ps[:], bcol(1), bcol(20), k1_sb[:], "k1e")
            v1_sb = opool.tile([128, 465], F16, name="v1_sb", tag="ovv")
            _lrelu(nc, misc, v1_ps[:], bcol(2), bcol(21), v1_sb[:], "v1e")
            k1o = k1_sb.rearrange("c (h w) -> c h w", h=15)
            v1o = v1_sb.rearrange("c (h w) -> c h w", h=15)

            # ============ k2/v2 partials + fp16 AllReduce ====================
            w2k = opool.tile([128, 2304], F16, name="w2k", tag="wk2")
            nc.gpsimd.dma_start(w2k[:], w2k_d.ap())
            kp = ppool.tile([128, 210], F32, name="kp", tag="pk")
            for cc in range(2):
                for tap in range(9):
                    dy, dx = tap // 3, tap % 3
                    wink = k1o[:, dy: dy + 13: 2, dx: dx + 29: 2]
                    nc.tensor.matmul(kp[:, 105 * cc:105 * cc + 105],
                                     w2k[:, tap * 256 + 128 * cc: tap * 256 + 128 * cc + 128],
                                     wink, start=(tap == 0), stop=(tap == 8))
            vp = ppool.tile([128, 1024], F32, name="vp", tag="pbig")
            for vh in range(2):
                w2v = wpool.tile([128, 4608], F16, name="w2v", tag="wv2")
                nc.gpsimd.dma_start(w2v[:], w2v_d.ap()[vh])
                for c4 in range(4):
                    cc = 4 * vh + c4
                    for tap in range(9):
                        dy, dx = tap // 3, tap % 3
                        winv = v1o[:, dy: dy + 13: 2, dx: dx + 29: 2]
                        nc.tensor.matmul(vp[:, 128 * cc:128 * cc + 105],
                                         w2v[:, tap * 512 + 128 * c4: tap * 512 + 128 * c4 + 128],
                                         winv, start=(tap == 0), stop=(tap == 8))
            with tc.high_priority():
                kps = misc.tile([128, 210], F16, name="kps", tag="rsk")
                nc.scalar.copy(kps[:], kp[:])
                vps = misc.tile([128, 840], F16, name="vps", tag="rsv")
                nc.vector.tensor_copy(vps.rearrange("p (c f) -> p c f", c=8),
                                      vp.rearrange("p (c f) -> p c f", c=8)[:, :, 0:105])
                nc.sync.dma_start(ar_in[0:26880].rearrange("(c p f) -> p c f",
                                                           c=2, p=128),
                                  kps.rearrange("p (c f) -> p c f", c=2))
                nc.sync.dma_start(ar_in[26880:134400].rearrange("(c p f) -> p c f",
                                                                c=8, p=128),
                                  vps.rearrange("p (c f) -> p c f", c=8))
                nc.gpsimd.collective_compute("AllReduce", ALU.add, replica_groups=RG,
                                             ins=[ar_in.opt()], outs=[ar_out.opt()])

            # k2/v2 epilogues on the reduced FULL tensors (overlap q1)
            k2r = misc.tile([128, 210], F16, name="k2r", tag="rsk2")
            nc.sync.dma_start(k2r.rearrange("p (c f) -> p c f", c=2),
                              ar_out[0:26880].rearrange("(c p f) -> p c f", c=2, p=128))
            v2r = misc.tile([128, 840], F16, name="v2r", tag="rsv2")
            nc.sync.dma_start(v2r.rearrange("p (c f) -> p c f", c=8),
                              ar_out[26880:134400].rearrange("(c p f) -> p c f", c=8, p=128))
            k2_sb = opool.tile([128, 210], F16, name="k2_sb", tag="okv2")
            for cc in range(2):
                _lrelu(nc, misc, k2r[:, 105 * cc:105 * cc + 105], bcol(5 + cc),
                       bcol(24 + cc), k2_sb[:, 105 * cc:105 * cc + 105], f"k2e{cc}")
            v2_sb = opool.tile([128, 840], F16, name="v2_sb", tag="ovv2")
            for cc in range(8):
                _lrelu(nc, misc, v2r[:, 105 * cc:105 * cc + 105], bcol(7 + cc),
                       bcol(26 + cc), v2_sb[:, 105 * cc:105 * cc + 105], f"v2e{cc}")

            # ============ q1 conv ============================================
            q1_ps = ppool.tile([128, 2048], F32, name="q1_ps", tag="pbig")
            for ic in range(16):
                wq = wpool.tile([128, 1152], F16, name="wq", tag="wA")
                nc.gpsimd.dma_start(wq[:], w1q_d.ap()[ic])
                x3 = xall[:, 2244 * ic:2244 * (ic + 1)].rearrange(
                    "c (h w) -> c h w", h=34)
                first, last = (ic == 0), (ic == 15)
                for tap in range(9):
                    dy, dx = tap // 3, tap % 3
                    wq_t = wq[:, tap * 128:tap * 128 + 128]
                    for t in range(4):
                        win = x3[:, 8 * t + dy: 8 * t + dy + 8, dx: dx + 64]
                        nc.tensor.matmul(q1_ps[:, 512 * t: 512 * t + 512], wq_t, win,
                                         start=(first and tap == 0),
                                         stop=(last and tap == 8))

            q1_sb = opool.tile([128, 34 * 66], F16, name="q1_sb", tag="obig")
            q1o = q1_sb.rearrange("c (h w) -> c h w", h=34)
            q1v = q1_ps.rearrange("c (h w) -> c h w", h=32)
            _lrelu(nc, misc, q1v[:, 0:16, :], bcol(0), bcol(19), q1o[:, 1:17, 1:65], "q1e0")
            _lrelu(nc, misc, q1v[:, 16:32, :], bcol(0), bcol(19), q1o[:, 17:33, 1:65], "q1e1")
            nc.vector.tensor_copy(q1o[:, 0:1, 1:65], q1o[:, 2:3, 1:65])
            nc.vector.tensor_copy(q1o[:, 33:34, 1:65], q1o[:, 31:32, 1:65])
            nc.vector.tensor_copy(q1o[:, :, 0:1], q1o[:, :, 64:65])
            nc.vector.tensor_copy(q1o[:, :, 65:66], q1o[:, :, 1:2])

            # ============ k3/v3 convs (channel-sharded) + tiny AllGather =====
            k3_ps = ppool.tile([32, 65], F32, name="k3_ps", tag="pk")
            for jc in range(2):
                k2c3 = k2_sb[:, 105 * jc:105 * jc + 105].rearrange(
                    "c (h w) -> c h w", h=7)
                w3k = wpool.tile([128, 288], F16, name="w3k", tag="wB")
                nc.gpsimd.dma_start(w3k[:], w3k_d.ap()[jc])
                for tap in range(9):
                    dy, dx = tap // 3, tap % 3
                    wink = k2c3[:, dy: dy + 5, dx: dx + 13]
                    nc.tensor.matmul(k3_ps[:], w3k[:, tap * 32:tap * 32 + 32], wink,
                                     start=(jc == 0 and tap == 0), stop=(jc == 1 and tap == 8))
            v3_ps = ppool.tile([128, 65], F32, name="v3_ps", tag="pv")
            for ic in range(8):
                v2c3 = v2_sb[:, 105 * ic:105 * ic + 105].rearrange(
                    "c (h w) -> c h w", h=7)
                w3v = wpool.tile([128, 1152], F16, name="w3v", tag="wC")
                nc.gpsimd.dma_start(w3v[:], w3v_d.ap()[ic])
                for tap in range(9):
                    dy, dx = tap // 3, tap % 3
                    winv = v2c3[:, dy: dy + 5, dx: dx + 13]
                    nc.tensor.matmul(v3_ps[:], w3v[:, tap * 128:tap * 128 + 128], winv,
                                     start=(ic == 0 and tap == 0), stop=(ic == 7 and tap == 8))

            k3_sb = misc.tile([32, 65], F16, name="k3_sb")
            _lrelu(nc, misc, k3_ps[:], bcol(17)[0:32], bcol(36)[0:32], k3_sb[:], "k3e")
            v3_sb = misc.tile([128, 65], F16, name="v3_sb")
            _lrelu(nc, misc, v3_ps[:], bcol(18), bcol(37), v3_sb[:], "v3e")
            nc.sync.dma_start(agkv_in[0:2080].rearrange("(p f) -> p f", p=32), k3_sb[:])
            nc.sync.dma_start(agkv_in[2080:10400].rearrange("(p f) -> p f", p=128), v3_sb[:])
            nc.gpsimd.collective_compute("AllGather", ALU.bypass, replica_groups=RG,
                                         ins=[agkv_in.opt()], outs=[agkv_out.opt()])

            # ============ q2 partials -> 2x fp16 AllToAll over position blocks
            w2q = opool.tile([128, 2304], F16, name="w2q", tag="wq2")
            nc.gpsimd.dma_start(w2q[:], w2q_d.ap())
            wpt = opool.tile([128, 8192], F16, name="wpt", tag="wpt")
            for i in range(8):
                nc.gpsimd.dma_start(wpt[:, 1024 * i:1024 * i + 1024], wp_d.ap()[i])
            w3qs = []
            for icc in range(2):
                w3q = opool.tile([128, 2304], F16, name="w3q", tag=f"wq3{icc}")
                nc.gpsimd.dma_start(w3q[:], w3q_d.ap()[icc])
                w3qs.append(w3q)

            engs = [nc.scalar.copy,
                    lambda o, i: nc.vector.tensor_copy(o, i)]
            NE = len(engs)
            for cc in range(2):
                qp = ppool.tile([128, 2048], F32, name="qp", tag="pbig")
                for tap in range(9):
                    dy, dx = tap // 3, tap % 3
                    wslc = w2q[:, tap * 256 + 128 * cc: tap * 256 + 128 * cc + 128]
                    for t in range(4):
                        win = q1o[:, 8 * t + dy: 8 * t + dy + 8, dx: dx + 64]
                        nc.tensor.matmul(qp[:, 512 * t:512 * t + 512], wslc, win,
                                         start=(tap == 0), stop=(tap == 8))
                # stage position slices (rows 4j-1 .. 4j+5, reflect at edges)
                stg = opool.tile([128, 3072], F16, name=f"stg{cc}", tag=f"stg{cc}")
                ei = 0
                for j in range(8):
                    base = 384 * j
                    if j == 0:
                        engs[ei % NE](stg[:, base:base + 64], qp[:, 64:128]); ei += 1
                        engs[ei % NE](stg[:, base + 64:base + 384], qp[:, 0:320]); ei += 1
                    elif j == 7:
                        engs[ei % NE](stg[:, base:base + 320], qp[:, 1728:2048]); ei += 1
                        engs[ei % NE](stg[:, base + 320:base + 384], qp[:, 1920:1984]); ei += 1
                    else:
                        engs[ei % NE](stg[:, base:base + 384],
                                      qp[:, 256 * j - 64:256 * j + 320]); ei += 1
                for j in range(8):
                    eng = nc.sync if j % 2 else nc.gpsimd
                    eng.dma_start(
                        a2a_in[j, 49152 * cc:49152 * cc + 49152].rearrange(
                            "(p f) -> p f", p=128),
                        stg[:, 384 * j:384 * j + 384])
            nc.gpsimd.collective_compute("AllToAll", ALU.bypass, replica_groups=RG,
                                         ins=[a2a_in.opt()], outs=[a2a_out.opt()])

            # v3/k3 full (from the kv AllGather) + Yt = v3^T wp, all overlapping
            # the AllToAlls. Yt folds the 1x1 projection through the attention
            # values so the post-softmax work is just one [65]-contraction.
            v3f = misc.tile([128, 520], F16, name="v3f", tag="v3f")
            for i in range(8):
                nc.sync.dma_start(v3f[:, 65 * i:65 * i + 65],
                                  agkv_out[i, 2080:10400].rearrange(
                                      "(p f) -> p f", p=128))
            k3f = misc.tile([128, 130], F16, name="k3f", tag="k3f")
            for mm in range(2):
                for rr in range(4):
                    nc.sync.dma_start(
                        k3f[32 * rr:32 * rr + 32, 65 * mm:65 * mm + 65],
                        agkv_out[4 * mm + rr, 0:2080].rearrange(
                            "(p f) -> p f", p=32))
            yt_ps = ppool.tile([65, 1024], F32, name="yt_ps", tag="pyt")
            for i in range(8):
                for h in range(2):
                    nc.tensor.matmul(yt_ps[:, 512 * h:512 * h + 512],
                                     v3f[:, 65 * i:65 * i + 65],
                                     wpt[:, 1024 * i + 512 * h:1024 * i + 512 * h + 512],
                                     start=(i == 0), stop=(i == 7))
            ytf = misc.tile([65, 1024], F16, name="ytf", tag="ytf")
            nc.scalar.copy(ytf[:], yt_ps[:])

            # receive + local 8-way reduce + epilogue, per cc chunk
            q2_sb = opool.tile([128, 792], F16, name="q2_sb", tag="oq2")
            dmaengs = [[nc.sync, nc.scalar], [nc.gpsimd, nc.sync]]
            recvs, q2rs = [], []
            for c in range(2):
                recvc = misc.tile([128, 3072], F16, name=f"recv{c}", tag=f"recv{c}")
                rv = recvc.rearrange("p (s f) -> p s f", s=8)
                for h in range(2):
                    dmaengs[c][h].dma_start(
                        rv[:, 4 * h:4 * h + 4],
                        a2a_out[4 * h:4 * h + 4,
                                49152 * c:49152 * c + 49152].rearrange(
                            "s (p f) -> p s f", p=128))
                recvs.append(recvc)
                q2rs.append(misc.tile([128, 384], F32, name=f"q2r{c}",
                                      tag=f"q2r{c}"))
            # c=0: vector strided reduce; c=1: gpsimd add tree (runs in parallel)
            nc.vector.reduce_sum(
                q2rs[0][:], recvs[0].rearrange("p (s f) -> p f s", s=8), axis=AX.X)
            nc.vector.reduce_sum(
                q2rs[1][:], recvs[1].rearrange("p (s f) -> p f s", s=8), axis=AX.X)
            for c in range(2):
                q2rg = q2rs[c].rearrange("c (h w) -> c h w", h=6)
                q2o = q2_sb[:, 396 * c:396 * c + 396].rearrange(
                    "c (h w) -> c h w", h=6)
                _lrelu(nc, misc, q2rg[:], bcol(3 + c), bcol(22 + c),
                       q2o[:, :, 1:65], f"q2e{c}")
                nc.vector.tensor_copy(q2o[:, :, 0:1], q2o[:, :, 64:65])
                nc.vector.tensor_copy(q2o[:, :, 65:66], q2o[:, :, 1:2])

            # ============ q3 conv (all 256 ch, own 4 rows) ===================
            q3_ps = ppool.tile([128, 1024], F32, name="q3_ps", tag="pbig")
            for icc in range(2):
                q2c3 = q2_sb[:, 396 * icc:396 * icc + 396].rearrange(
                    "c (h w) -> c h w", h=6)
                for oc in range(2):
                    for tap in range(9):
                        dy, dx = tap // 3, tap % 3
                        win = q2c3[:, dy: dy + 4, dx: dx + 64]
                        nc.tensor.matmul(q3_ps[:, 512 * oc:512 * oc + 256],
                                         w3qs[icc][:, tap * 256 + 128 * oc: tap * 256 + 128 * oc + 128],
                                         win, start=(icc == 0 and tap == 0),
                                         stop=(icc == 1 and tap == 8))
            q3_sb = misc.tile([128, 512], F16, name="q3_sb", tag="q3sb")
            for oc in range(2):
                _lrelu(nc, misc, q3_ps[:, 512 * oc:512 * oc + 256], bcol(15 + oc),
                       bcol(34 + oc), q3_sb[:, 256 * oc:256 * oc + 256], f"q3e{oc}")

            # ============ scores + softmax stats =============================
            sc_ps = ppool.tile([65, 256], F32, name="sc_ps", tag="pk")
            for icc in range(2):
                nc.tensor.matmul(sc_ps[:], k3f[:, 65 * icc:65 * icc + 65],
                                 q3_sb[:, 256 * icc:256 * icc + 256],
                                 start=(icc == 0), stop=(icc == 1))
            negmax = misc.tile([65, 1], F32, name="negmax", tag="negmax")
            nc.vector.reduce_max(negmax[:], sc_ps[:], axis=AX.X, negate=True)
            esum = misc.tile([65, 1], F32, name="esum", tag="esum")
            bexp = misc.tile([65, 256], F16, name="bexp", tag="bexp")
            nc.scalar.activation(bexp[:], sc_ps[:], AF.Exp, bias=negmax[:, 0:1],
                                 accum_out=esum[:, 0:1])
            stats = misc.tile([65, 2], F32, name="stats", tag="stats")
            nc.vector.tensor_scalar_mul(stats[:, 0:1], negmax[:], -1.0)
            nc.vector.tensor_copy(stats[:, 1:2], esum[:])
            nc.sync.dma_start(st_in[0:130].rearrange("(p c) -> p c", p=65), stats[:])
            nc.gpsimd.collective_compute("AllGather", ALU.bypass, replica_groups=RG,
                                         ins=[st_in.opt()], outs=[st_out.opt()])
            st8 = misc.tile([65, 16], F32, name="st8", tag="st8")
            nc.sync.dma_start(st8.rearrange("p (r c) -> p r c", r=8),
                              st_out[:].rearrange("r (p c) -> p r c", p=65))
            mstar = misc.tile([65, 1], F32, name="mstar", tag="mstar")
            nc.vector.reduce_max(mstar[:], st8[:, 0:16:2], axis=AX.X)
            diff9 = misc.tile([65, 9], F32, name="diff9", tag="diff9")
            nc.vector.tensor_scalar_sub(diff9[:, 0:8], st8[:, 0:16:2], mstar[:, 0:1])
            nc.vector.tensor_tensor(diff9[:, 8:9], stats[:, 0:1], mstar[:],
                                    op=ALU.subtract)
            nc.scalar.activation(diff9[:], diff9[:], AF.Exp)
            texp = misc.tile([65, 8], F32, name="texp", tag="texp")
            nc.vector.tensor_tensor(texp[:], diff9[:, 0:8], st8[:, 1:16:2],
                                    op=ALU.mult)
            dsum = misc.tile([65, 1], F32, name="dsum", tag="dsum")
            nc.vector.reduce_sum(dsum[:], texp[:], axis=AX.X)
            rd = misc.tile([65, 1], F32, name="rd", tag="rd")
            nc.vector.reciprocal(rd[:], dsum[:])
            g = misc.tile([65, 1], F32, name="g", tag="g")
            nc.vector.tensor_tensor(g[:], diff9[:, 8:9], rd[:], op=ALU.mult)

            # ============ output: Yt^T [65] contraction with (g * bexp) ======
            nc.vector.tensor_scalar_mul(bexp[:], bexp[:], g[:, 0:1])
            for j in range(8):
                out_ps = ppool.tile([128, 256], F32, name="out_ps",
                                    tag=["pv", "pk", "pbig"][j % 3])
                nc.tensor.matmul(out_ps[:], ytf[:, 128 * j:128 * j + 128], bexp[:],
                                 start=True, stop=True)
                out_sb = misc.tile([128, 256], F16, name="out_sb", tag="osb", bufs=2)
                nc.vector.tensor_scalar_add(out_sb[:], out_ps[:], bcol(38 + j))
                nc.sync.dma_start(out_d.ap()[128 * j:128 * j + 128], out_sb[:])

    nc.compile()
    nc.m = get_hw_module(nc.m)
    return nc


def _prep_inputs(x, qw1, qb1, qw2, qb2, qw3, qb3, kw1, kb1, kw2, kb2, kw3, kb3,
                 vw1, vb1, vw2, vb2, vw3, vb3, pw, pb):
    f16 = np.float16
    x = np.ascontiguousarray(np.asarray(x).reshape(2048, 32, 64), dtype=np.float32)
    xp = np.concatenate([x[:, 1:2], x, x[:, 30:31]], axis=1)
    xp = np.concatenate([xp[:, :, -1:], xp, xp[:, :, :1]], axis=2)
    xpad = np.ascontiguousarray(xp.reshape(16, 128, 34 * 66), dtype=f16)

    def conv_w(wt, co_lo, co_n, nchunk):
        ws = np.asarray(wt)[co_lo:co_lo + co_n]           # (co_n, Ci, 3, 3)
        ci = ws.shape[1]
        a = ws.reshape(co_n, nchunk, ci // nchunk, 9)     # (co, ck, ci, tap)
        a = a.transpose(1, 2, 3, 0)                       # (ck, ci, tap, co)
        return np.ascontiguousarray(a.reshape(nchunk, ci // nchunk, 9 * co_n),
                                    dtype=f16)

    def conv_w_ci(wt, ci_lo):
        # full out-channels, my 128 input channels -> (128ci, 9*co)
        ws = np.asarray(wt)[:, ci_lo:ci_lo + 128]         # (co, 128, 3, 3)
        co = ws.shape[0]
        a = ws.reshape(co, 128, 9).transpose(1, 2, 0)     # (ci, tap, co)
        return np.ascontiguousarray(a.reshape(128, 9 * co), dtype=f16)

    # full q3 weights: (2 ic-chunk, 128 ci, 9 tap * 256 co)
    w3q_full = np.asarray(qw3).reshape(256, 2, 128, 9).transpose(1, 2, 3, 0)
    w3q_full = np.ascontiguousarray(w3q_full.reshape(2, 128, 2304), dtype=f16)

    in_maps = []
    for c in range(NCORES):
        m = {"xpad": xpad}
        m["w1q"] = conv_w(qw1, 128 * c, 128, 16)
        m["w1k"] = conv_w(kw1, 128 * c, 128, 16)
        m["w1v"] = conv_w(vw1, 128 * c, 128, 16)
        m["w2q"] = conv_w_ci(qw2, 128 * c)
        m["w2k"] = conv_w_ci(kw2, 128 * c)
        wv2 = np.asarray(vw2)[:, 128 * c:128 * c + 128]        # (1024co, 128ci, 3, 3)
        wv2 = wv2.reshape(2, 512, 128, 9).transpose(0, 2, 3, 1)  # (half, ci, tap, co512)
        m["w2v"] = np.ascontiguousarray(wv2.reshape(2, 128, 4608), dtype=f16)
        m["w3q"] = w3q_full
        m["w3k"] = conv_w(kw3, 32 * c, 32, 2)
        m["w3v"] = conv_w(vw3, 128 * c, 128, 8)
        m["wp"] = np.ascontiguousarray(
            np.asarray(pw)[:, :, 0, 0].T.reshape(8, 128, 1024), dtype=f16)
        bias = np.zeros((128, 46), np.float32)
        bias[:, 0] = qb1[128 * c:128 * c + 128]
        bias[:, 1] = kb1[128 * c:128 * c + 128]
        bias[:, 2] = vb1[128 * c:128 * c + 128]
        bias[:, 3] = qb2[0:128]
        bias[:, 4] = qb2[128:256]
        bias[:, 5] = kb2[0:128]
        bias[:, 6] = kb2[128:256]
        for i in range(8):
            bias[:, 7 + i] = vb2[128 * i:128 * i + 128]
        bias[:, 15] = qb3[0:128]
        bias[:, 16] = qb3[128:256]
        bias[0:32, 17] = kb3[32 * c:32 * c + 32]
        bias[:, 18] = vb3[128 * c:128 * c + 128]
        bias[:, 19:38] = 0.3 * bias[:, 0:19]
        for j in range(8):
            bias[:, 38 + j] = pb[128 * j:128 * j + 128]
        m["bias"] = bias
        in_maps.append(m)
    return in_maps


LAST_RESULT = None


def kernel(**inputs):
    global LAST_RESULT
    if "nc" not in _CACHE:
        _CACHE["nc"] = build_program()
    nc = _CACHE["nc"]
    in_maps = _prep_inputs(**{k: np.asarray(v) for k, v in inputs.items()})
    res = bass_utils.run_bass_kernel_spmd(nc, in_maps, core_ids=list(range(NCORES)))
    LAST_RESULT = res
    out = np.empty((1024, 32, 64), np.float32)
    for c in range(NCORES):
        out[:, 4 * c:4 * c + 4, :] = res.results[c]["out_shard"].astype(
            np.float32).reshape(1024, 4, 64)
    return np.ascontiguousarray(out.reshape(1, 1024, 32, 64))
